# revision 1
# baseline (speedup 1.0000x reference)
"""Fused decoder-layer kernel for one TRN2 chip (8 NeuronCores).

Problem (B=2, S=2048, DIM=1024, H=16, DH=64, DFF=2048):
    h1 = MHA(q=de_x, k=de_x, v=de_x, mask)   (shared per-head weights Wq/Wk/Wv)
    h2 = MHA(q=en_x, k=en_x, v=h1,  None)
    y  = relu(h2 @ W1 + b1) @ W2 + b2

Sharding: core c = 4*b + g  (b = batch, g = head-group of 4 heads; g also
indexes the S/4 slice of rows this core runs the FFN on).

Layout strategy (all matmul operands pre-transposed so every contraction has
its reduction dim on SBUF partitions; all matmul inputs bf16, PSUM f32):
  - host passes x^T [DIM, S] per batch; projections produce q^T/k^T [DH, S]
    and v [S, DH] directly.
  - scores are computed transposed ([s2, s1]) so P^T feeds the PV matmul as
    stationary weights with K = s2 on partitions.
  - a ones-column appended to v makes the PV matmul also produce softmax row
    sums (flash-attention style); normalization happens on the [DH, S] output.
  - FFN runs column-transposed (ff1^T, y^T) so b1/b2 are per-partition ACT
    biases and no transposes are ever needed. Host un-transposes y^T.
Cross-core:
  - h1^T shards are AllGather'd within each batch group (v2 projection needs
    the full feature dim).
  - h2^T is exchanged with one 8-way AllToAll so each core ends up with its
    fixed S/4 column slice (SPMD program has no per-core offsets). Shards
    from the other batch land in known row blocks and are zeroed via a
    per-core row-mask input before the FFN contraction.
"""

import math

import numpy as np
import ml_dtypes

import concourse.bass as bass
import concourse.bacc as bacc
import concourse.mybir as mybir
import concourse.tile as tile
import concourse.bass_utils as bass_utils

B, S, DIM, H = 2, 2048, 1024, 16
DH = DIM // H            # 64
DFF = 2048
NEG = -1.0e9
N_CORES = 8
G = 4                    # cores per batch group == head-groups == s-groups
HPC = H // G             # heads per core = 4
SL = S // G              # FFN rows per core = 512
KC = DIM // 128          # 8 k-chunks of the model dim
NB = S // 128            # 16 key blocks
NT = S // 1024           # 2 query tiles of 1024
BF16 = mybir.dt.bfloat16
F32 = mybir.dt.float32
AF = mybir.ActivationFunctionType

_CACHE: dict = {}


def _mask_plan(mask: np.ndarray):
    """Classify each [1024 x 128] (s1, s2) block: 'N' no-op, 'M' apply, 'S' skip."""
    plan = []
    for t in range(NT):
        row = []
        for blk in range(NB):
            sub = mask[t * 1024:(t + 1) * 1024, blk * 128:(blk + 1) * 128]
            # NOTE: an 'S' (skip-block) fast path deadlocked on hardware;
            # fully-masked blocks are handled as 'M' (exp underflows to 0).
            row.append('N' if not sub.any() else 'M')
        plan.append(tuple(row))
    return tuple(plan)


def _build(plan):
    has_mask = any(c == 'M' for row in plan for c in row)
    nc = bacc.Bacc("TRN2", target_bir_lowering=False, debug=False,
                   num_devices=N_CORES)

    de_xT = nc.dram_tensor("de_xT", [DIM, S], BF16, kind="ExternalInput")
    en_xT = nc.dram_tensor("en_xT", [DIM, S], BF16, kind="ExternalInput")
    wq = nc.dram_tensor("wq", [DIM, HPC * DH], BF16, kind="ExternalInput")
    wk = nc.dram_tensor("wk", [DIM, HPC * DH], BF16, kind="ExternalInput")
    wv = nc.dram_tensor("wv", [DIM, HPC * DH], BF16, kind="ExternalInput")
    w1 = nc.dram_tensor("w1", [DIM, DFF], BF16, kind="ExternalInput")
    w2 = nc.dram_tensor("w2", [DFF, DIM], BF16, kind="ExternalInput")
    b1t = nc.dram_tensor("b1t", [128, DFF // 128], F32, kind="ExternalInput")
    b2t = nc.dram_tensor("b2t", [128, DIM // 128], F32, kind="ExternalInput")
    zmask = nc.dram_tensor("zmask", [128, 2 * KC], F32, kind="ExternalInput")
    maskT = None
    if has_mask:
        maskT = nc.dram_tensor("maskT", [S, S], BF16, kind="ExternalInput")
    yT = nc.dram_tensor("yT", [DIM, SL], F32, kind="ExternalOutput")

    with tile.TileContext(nc) as tc:
        _trace(nc, tc, plan, de_xT, en_xT, wq, wk, wv, w1, w2, b1t, b2t,
               zmask, maskT, yT)
    nc.compile()
    return nc, has_mask


def _trace(nc, tc, plan, de_xT, en_xT, wq, wk, wv, w1, w2, b1t, b2t,
           zmask, maskT, yT):
    # Unified allocation stack: everything (pools and single tiles) must be
    # released in strict LIFO order before TileContext exits.
    stack = nc._tile_stack = []   # [release_fn or None(done)]

    def _push(release_fn):
        ent = {"f": release_fn}
        stack.append(ent)
        def rel():
            assert ent["f"] is not None, "double release"
            ent["f"](); ent["f"] = None
        return rel

    def release_rest():
        for ent in reversed(stack):
            if ent["f"] is not None:
                ent["f"](); ent["f"] = None

    noplan = tuple(tuple('N' for _ in range(NB)) for _ in range(NT))

    # ---- pools ----------------------------------------------------------
    def pool(**kw):
        cm = tc.tile_pool(**kw)
        p = cm.__enter__()
        _push(lambda: cm.__exit__(None, None, None))
        return p

    def single(shape, dtype, name):
        t_, f_ = tc.tile(shape, dtype, name=name)
        return t_, _push(f_)

    ps_big = pool(name="ps_big", bufs=2, space="PSUM")
    ps_hext = pool(name="ps_hext", bufs=2, space="PSUM")
    pt_pool = pool(name="pt", bufs=6)
    rc_pool = pool(name="rc", bufs=3)
    bc_pool = pool(name="bc", bufs=3)
    q_pool = pool(name="qp", bufs=2)
    k_pool = pool(name="kp", bufs=2)
    v_pool = pool(name="vp", bufs=1)
    sh_pool = pool(name="shp", bufs=2)
    y_pool = pool(name="yp", bufs=2)
    dram = pool(name="dram", bufs=1, space="DRAM")

    # ---- persistent tiles (stack order: frees must pop LIFO) ------------
    wq_sb, _ = single([128, KC, HPC * DH], BF16, "wqsb")
    wk_sb, _ = single([128, KC, HPC * DH], BF16, "wksb")
    wv_sb, _ = single([128, KC, HPC * DH], BF16, "wvsb")
    # en allocated BELOW de on the stack: de is freed first (after attn1),
    # en later (after attn2 q/k projections).
    en_sb, en_free = [], []
    for kc in range(KC):
        t_, f_ = single([128, S], BF16, f"en{kc}")
        en_sb.append(t_); en_free.append(f_)
    de_sb, de_free = [], []
    for kc in range(KC):
        t_, f_ = single([128, S], BF16, f"de{kc}")
        de_sb.append(t_); de_free.append(f_)
    # DMA trace order: attn1's operands first
    nc.sync.dma_start(wq_sb[:], wq.rearrange("(a p) c -> p a c", p=128))
    nc.sync.dma_start(wk_sb[:], wk.rearrange("(a p) c -> p a c", p=128))
    for kc in range(KC):
        nc.sync.dma_start(de_sb[kc][:], de_xT[kc * 128:(kc + 1) * 128, :])
    nc.sync.dma_start(wv_sb[:], wv.rearrange("(a p) c -> p a c", p=128))
    for kc in range(KC):
        nc.sync.dma_start(en_sb[kc][:], en_xT[kc * 128:(kc + 1) * 128, :])

    # split collective bounce buffers: AG1 per query-tile half, A2A per
    # head-pair half — each half fires as soon as its producer loop is done,
    # so the wire time hides under the remaining attention compute.
    cc1_in = [dram.tile([2 * 128, 1024], BF16, name=f"cc1i{t}")
              for t in range(NT)]
    cc1_out = [dram.tile([KC * 128, 1024], BF16, name=f"cc1o{t}")
               for t in range(NT)]
    cc2_in = [dram.tile([KC * 128, SL], BF16, name=f"cc2i{p}")
              for p in range(2)]
    cc2_out = [dram.tile([KC * 128, SL], BF16, name=f"cc2o{p}")
               for p in range(2)]

    # ---- helpers --------------------------------------------------------
    def project_qk_pair(x_sb, w_sb, pool, pair):
        """q^T (or k^T) for one head pair as a [128, S] bf16 tile."""
        qt = pool.tile([128, S], BF16, tag="qk", name=f"qk{pair}")
        for st in range(NT):
            ps = ps_big.tile([128, 1024], F32, tag="ps", name="pjps")
            for kc in range(KC):
                for nn in (0, 512):
                    nc.tensor.matmul(
                        ps[:, nn:nn + 512],
                        w_sb[:, kc, pair * 128:(pair + 1) * 128],
                        x_sb[kc][:, st * 1024 + nn:st * 1024 + nn + 512],
                        start=(kc == 0), stop=(kc == KC - 1))
            nc.vector.tensor_copy(qt[:, st * 1024:(st + 1) * 1024], ps[:])
        return qt

    def project_v(src_sb, v_all, blk_lo, blk_hi):
        """v for 4 heads + ones column into v_all[128, NB*HPC*65] (bf16)."""
        for blk in range(blk_lo, blk_hi):
            ps = ps_big.tile([128, 1024], F32, tag="ps", name="vps")
            for kc in range(KC):
                nc.tensor.matmul(
                    ps[:, 0:HPC * DH],
                    src_sb[kc][:, blk * 128:(blk + 1) * 128],
                    wv_sb[:, kc, :],
                    start=(kc == 0), stop=(kc == KC - 1))
            for h in range(HPC):
                nc.vector.tensor_copy(
                    v_all[:, (blk * HPC + h) * 65:(blk * HPC + h) * 65 + 64],
                    ps[:, h * DH:(h + 1) * DH])

    def new_v_all():
        v_all = v_pool.tile([128, NB * HPC * 65], BF16, tag="v", name="vall")
        v3 = v_all[:].rearrange("p (b c) -> p b c", c=65)
        nc.vector.memset(v3[:, :, 64:65], 1.0)
        return v_all

    def attn_unit(q_pairs, k_pairs, v_all, aplan, mask_tiles_in, sh, h, t):
        """scores -> exp -> PV -> normalize for one (head, query-tile)."""
        pair, off = h // 2, (h % 2) * 64
        hext = ps_hext.tile([65, 1024], F32, name="hext")
        for blk in range(NB):
            sc = ps_big.tile([128, 1024], F32, tag="ps", name="scps")
            for nn in (0, 512):
                nc.tensor.matmul(
                    sc[:, nn:nn + 512],
                    k_pairs[pair][off:off + 64, blk * 128:(blk + 1) * 128],
                    q_pairs[pair][off:off + 64,
                                  t * 1024 + nn:t * 1024 + nn + 512],
                    start=True, stop=True)
            if aplan[t][blk] == 'M':
                nc.vector.tensor_add(sc[:], sc[:], mask_tiles_in[(t, blk)][:])
            pt = pt_pool.tile([128, 1024], BF16, name="pt")
            nc.scalar.activation(pt[:], sc[:], AF.Exp)
            vs = v_all[:, (blk * HPC + h) * 65:(blk * HPC + h) * 65 + 65]
            for nn in (0, 512):
                nc.tensor.matmul(
                    hext[:, nn:nn + 512], vs, pt[:, nn:nn + 512],
                    start=(blk == 0), stop=(blk == NB - 1))
        recip = rc_pool.tile([1, 1024], F32, name="recip")
        nc.vector.reciprocal(recip[:], hext[64:65, :])
        rbc = bc_pool.tile([64, 1024], F32, name="rbc")
        nc.gpsimd.partition_broadcast(rbc[:], recip[0:1, :])
        nc.vector.tensor_mul(
            sh[pair][off:off + 64, t * 1024:(t + 1) * 1024],
            hext[0:64, :], rbc[:])

    def new_sh():
        return [sh_pool.tile([128, S], BF16, tag="sh", name=f"sh{p}")
                for p in range(2)]

    # ---- attention 1 (self-attn on de_x, mask) --------------------------
    # pair-0 q/k and v project first so the exp pipeline (ACT) starts as
    # early as possible; pair-1 projections slot in behind the first units.
    q1 = [None, None]
    k1 = [None, None]
    q1[0] = project_qk_pair(de_sb, wq_sb, q_pool, 0)
    k1[0] = project_qk_pair(de_sb, wk_sb, k_pool, 0)
    v1 = new_v_all()
    project_v(de_sb, v1, 0, NB)
    mask_tiles, mask_free = {}, []
    for t in range(NT):
        for blk in range(NB):
            if plan[t][blk] == 'M':
                mt, fm = single([128, 1024], BF16, f"mk{t}_{blk}")
                nc.sync.dma_start(
                    mt[:], maskT[blk * 128:(blk + 1) * 128,
                                 t * 1024:(t + 1) * 1024])
                mask_tiles[(t, blk)] = mt
                mask_free.append(fm)
    # t-major so each query-tile half of h1^T AllGathers while the other half
    # is still computing
    h1sh = new_sh()
    for t in range(NT):
        for h in range(HPC):
            if t == 0 and h == 2:
                q1[1] = project_qk_pair(de_sb, wq_sb, q_pool, 1)
                k1[1] = project_qk_pair(de_sb, wk_sb, k_pool, 1)
            attn_unit(q1, k1, v1, plan, mask_tiles, h1sh, h, t)
        for p in range(2):
            nc.sync.dma_start(cc1_in[t][p * 128:(p + 1) * 128, :],
                              h1sh[p][:, t * 1024:(t + 1) * 1024])
        nc.gpsimd.collective_compute(
            "AllGather", mybir.AluOpType.bypass,
            replica_groups=[[0, 1, 2, 3], [4, 5, 6, 7]],
            ins=[cc1_in[t].opt()], outs=[cc1_out[t].opt()])
    for f in reversed(mask_free):
        f()
    for f in reversed(de_free):
        f()

    # ---- attention 2 (q,k from en_x; v from gathered h1) ----------------
    q2 = [None, None]
    k2 = [None, None]
    q2[0] = project_qk_pair(en_sb, wq_sb, q_pool, 0)
    k2[0] = project_qk_pair(en_sb, wk_sb, k_pool, 0)
    q2[1] = project_qk_pair(en_sb, wq_sb, q_pool, 1)
    k2[1] = project_qk_pair(en_sb, wk_sb, k_pool, 1)
    h1T_sb, h1T_free = [], []
    for kc in range(KC):
        t_, f_ = single([128, S], BF16, f"h1T{kc}")
        h1T_sb.append(t_); h1T_free.append(f_)
    v2 = new_v_all()
    for t in range(NT):
        for kc in range(KC):
            nc.sync.dma_start(h1T_sb[kc][:, t * 1024:(t + 1) * 1024],
                              cc1_out[t][kc * 128:(kc + 1) * 128, :])
        project_v(h1T_sb, v2, t * NB // NT, (t + 1) * NB // NT)
    for f in reversed(h1T_free):
        f()
    for f in reversed(en_free):
        f()

    # FFN weights prefetch during attention 2
    w1_sb, w1_free = [], []
    for kc in range(KC):
        t_, f_ = single([128, DFF], BF16, f"w1_{kc}")
        nc.sync.dma_start(t_[:], w1[kc * 128:(kc + 1) * 128, :])
        w1_sb.append(t_); w1_free.append(f_)
    w2_sb, w2_free = [], []
    for dc in range(DFF // 128):
        t_, f_ = single([128, DIM], BF16, f"w2_{dc}")
        nc.sync.dma_start(t_[:], w2[dc * 128:(dc + 1) * 128, :])
        w2_sb.append(t_); w2_free.append(f_)
    b1_sb, _ = single([128, DFF // 128], F32, "b1sb")
    b2_sb, _ = single([128, DIM // 128], F32, "b2sb")
    zm_sb, _ = single([128, 2 * KC], F32, "zmsb")
    nc.sync.dma_start(b1_sb[:], b1t[:])
    nc.sync.dma_start(b2_sb[:], b2t[:])
    nc.sync.dma_start(zm_sb[:], zmask[:])

    # pair-major so each head-pair half of h2^T AllToAlls while the other
    # pair is still computing
    h2sh = new_sh()
    for pair in range(2):
        for hh in range(2):
            for t in range(NT):
                attn_unit(q2, k2, v2, noplan, {}, h2sh, pair * 2 + hh, t)
        for j in range(2 * G):
            nc.sync.dma_start(
                cc2_in[pair][j * 128:(j + 1) * 128, :],
                h2sh[pair][:, (j % G) * SL:(j % G + 1) * SL])
        nc.gpsimd.collective_compute(
            "AllToAll", mybir.AluOpType.bypass,
            replica_groups=[[0, 1, 2, 3, 4, 5, 6, 7]],
            ins=[cc2_in[pair].opt()], outs=[cc2_out[pair].opt()])

    # ---- FFN on own S/4 rows -------------------------------------------
    # The A2A delivers each feature row twice (once per batch group); zero the
    # foreign-batch copy via the zmask input, then fold the two copies
    # together so the FFN contraction only runs over the real 1024 rows.
    h2_sb, h2_free = [], []     # [pair][j], j in 0..G
    for pair in range(2):
        row = []
        for i in range(2 * G):
            t_, f_ = single([128, SL], BF16, f"h2_{pair}_{i}")
            nc.sync.dma_start(t_[:], cc2_out[pair][i * 128:(i + 1) * 128, :])
            nc.vector.tensor_scalar_mul(
                t_[:], t_[:], zm_sb[:, pair * 2 * G + i:pair * 2 * G + i + 1])
            row.append(t_); h2_free.append(f_)
        for j in range(G):
            nc.vector.tensor_add(row[j][:], row[j][:], row[j + G][:])
        h2_sb.append(row[:G])

    # pass A: pair-0 partial sums land in SBUF f32 while A2A-b is in flight
    fp_sb, fp_free = [], []
    for dffb in range(DFF // 128):
        ps = ps_big.tile([128, 1024], F32, tag="ps", name="fAps")
        for i in range(G):
            nc.tensor.matmul(
                ps[:, 0:SL],
                w1_sb[2 * i][:, dffb * 128:(dffb + 1) * 128],
                h2_sb[0][i][:],
                start=(i == 0), stop=(i == G - 1))
        t_, f_ = single([128, SL], BF16, f"fp_{dffb}")
        nc.vector.tensor_copy(t_[:], ps[:, 0:SL])
        fp_sb.append(t_); fp_free.append(f_)
    # pass B: pair-1 contribution + pass-A partial, relu, bias
    ff1_sb, ff1_free = [], []
    for dffb in range(DFF // 128):
        ps = ps_big.tile([128, 1024], F32, tag="ps", name="fBps")
        for i in range(G):
            nc.tensor.matmul(
                ps[:, 0:SL],
                w1_sb[2 * i + 1][:, dffb * 128:(dffb + 1) * 128],
                h2_sb[1][i][:],
                start=(i == 0), stop=(i == G - 1))
        nc.vector.tensor_add(ps[:, 0:SL], ps[:, 0:SL], fp_sb[dffb][:])
        t_, f_ = single([128, SL], BF16, f"ff1_{dffb}")
        nc.scalar.activation(t_[:], ps[:, 0:SL], AF.Relu,
                             bias=b1_sb[:, dffb:dffb + 1])
        ff1_sb.append(t_); ff1_free.append(f_)
    for dimb in range(DIM // 128):
        ps = ps_big.tile([128, 1024], F32, tag="ps", name="yps")
        for dc in range(DFF // 128):
            nc.tensor.matmul(
                ps[:, 0:SL],
                w2_sb[dc][:, dimb * 128:(dimb + 1) * 128],
                ff1_sb[dc][:],
                start=(dc == 0), stop=(dc == DFF // 128 - 1))
        ysb = y_pool.tile([128, SL], F32, tag="y", name="ysb")
        nc.vector.tensor_scalar_add(ysb[:], ps[:, 0:SL],
                                    b2_sb[:, dimb:dimb + 1])
        nc.sync.dma_start(yT[dimb * 128:(dimb + 1) * 128, :], ysb[:])

    release_rest()


def _prep_inputs(de_x, en_x, mask, Wq, Wk, Wv, W1, b1, W2, b2, has_mask):
    bf = ml_dtypes.bfloat16
    scale = 1.0 / math.sqrt(DH)
    in_maps = []
    deT = [np.ascontiguousarray(de_x[b].T).astype(bf) for b in range(B)]
    enT = [np.ascontiguousarray(en_x[b].T).astype(bf) for b in range(B)]
    w1b = W1.astype(bf)
    w2b = W2.astype(bf)
    b1t = np.ascontiguousarray(b1.reshape(DFF // 128, 128).T).astype(np.float32)
    b2t = np.ascontiguousarray(b2.reshape(DIM // 128, 128).T).astype(np.float32)
    mT = None
    if has_mask:
        mT = np.ascontiguousarray(mask.T * np.float32(NEG)).astype(bf)
    for c in range(N_CORES):
        b, g = divmod(c, G)
        hs = slice(g * HPC, (g + 1) * HPC)
        m = {
            "de_xT": deT[b],
            "en_xT": enT[b],
            "wq": np.ascontiguousarray(
                np.transpose(Wq[hs] * scale, (1, 0, 2)).reshape(DIM, HPC * DH)
            ).astype(bf),
            "wk": np.ascontiguousarray(
                np.transpose(Wk[hs], (1, 0, 2)).reshape(DIM, HPC * DH)).astype(bf),
            "wv": np.ascontiguousarray(
                np.transpose(Wv[hs], (1, 0, 2)).reshape(DIM, HPC * DH)).astype(bf),
            "w1": w1b, "w2": w2b, "b1t": b1t, "b2t": b2t,
        }
        zm = np.zeros((128, 2 * KC), np.float32)
        for pair in range(2):
            for i in range(2 * G):
                if i // G == b:
                    zm[:, pair * 2 * G + i] = 1.0
        m["zmask"] = zm
        if has_mask:
            m["maskT"] = mT
        in_maps.append(m)
    return in_maps


def get_program(mask):
    plan = _mask_plan(np.asarray(mask))
    if plan not in _CACHE:
        _CACHE[plan] = _build(plan)
    return _CACHE[plan]


_RUNNERS: dict = {}


def _fast_runner(nc):
    """Build (once) a cached jitted SPMD executor for this program.

    run_bass_kernel_spmd re-creates and re-traces its jax.jit closure on
    every call; caching the jitted shard_map shaves seconds of dispatch
    overhead off warm calls. Mirrors bass2jax.run_bass_via_pjrt.
    """
    import jax
    from jax.sharding import Mesh, PartitionSpec
    try:
        from jax.experimental.shard_map import shard_map
    except ImportError:
        from jax.shard_map import shard_map
    import concourse.mybir as _mb
    from concourse import bass2jax as b2j

    b2j.install_neuronx_cc_hook()
    partition_name = (nc.partition_id_tensor.name
                      if nc.partition_id_tensor else None)
    in_names, out_names, out_avals = [], [], []
    for alloc in nc.m.functions[0].allocations:
        if not isinstance(alloc, _mb.MemoryLocationSet):
            continue
        name = alloc.memorylocations[0].name
        if alloc.kind == "ExternalInput":
            if name != partition_name:
                in_names.append(name)
        elif alloc.kind == "ExternalOutput":
            out_names.append(name)
            out_avals.append(jax.core.ShapedArray(
                tuple(alloc.tensor_shape), _mb.dt.np(alloc.dtype)))
    n_params = len(in_names)
    n_outs = len(out_avals)
    all_names = in_names + out_names + ([partition_name] if partition_name else [])
    donate = tuple(range(n_params, n_params + n_outs))

    def _body(*args):
        operands = list(args)
        if partition_name is not None:
            operands.append(b2j.partition_id_tensor())
        return tuple(b2j._bass_exec_p.bind(
            *operands,
            out_avals=tuple(out_avals),
            in_names=tuple(all_names),
            out_names=tuple(out_names),
            lowering_input_output_aliases=(),
            sim_require_finite=True,
            sim_require_nnan=True,
            nc=nc,
        ))

    devices = jax.devices()[:N_CORES]
    mesh = Mesh(np.asarray(devices), ("core",))
    in_specs = (PartitionSpec("core"),) * (n_params + n_outs)
    out_specs = (PartitionSpec("core"),) * n_outs
    sharded = jax.jit(
        shard_map(_body, mesh=mesh, in_specs=in_specs, out_specs=out_specs,
                  check_rep=False),
        donate_argnums=donate, keep_unused=True)

    def runner(in_maps):
        concat_in = [np.concatenate([in_maps[c][nm] for c in range(N_CORES)],
                                    axis=0) for nm in in_names]
        zeros = [np.zeros((N_CORES * a.shape[0], *a.shape[1:]), a.dtype)
                 for a in out_avals]
        out_arrs = sharded(*concat_in, *zeros)
        return [
            {nm: np.asarray(out_arrs[i]).reshape(N_CORES, *out_avals[i].shape)[c]
             for i, nm in enumerate(out_names)}
            for c in range(N_CORES)
        ]

    return runner


def run(inputs, want_results=False, **run_kwargs):
    nc, has_mask = get_program(inputs["mask"])
    in_maps = _prep_inputs(
        inputs["de_x"], inputs["en_x"], inputs["mask"],
        inputs["Wq"], inputs["Wk"], inputs["Wv"],
        inputs["W1"], inputs["b1"], inputs["W2"], inputs["b2"], has_mask)
    results = None
    res = None
    if not run_kwargs:
        try:
            key = id(nc)
            if key not in _RUNNERS:
                _RUNNERS[key] = _fast_runner(nc)
            results = _RUNNERS[key](in_maps)
        except Exception:
            results = None
    if results is None:
        res = bass_utils.run_bass_kernel_spmd(
            nc, in_maps, core_ids=list(range(N_CORES)), **run_kwargs)
        results = res.results
    y = np.empty((B, S, DIM), np.float32)
    for c in range(N_CORES):
        b, g = divmod(c, G)
        y[b, g * SL:(g + 1) * SL, :] = results[c]["yT"].T
    return (y, res) if want_results else y


def kernel(**inputs) -> np.ndarray:
    return run({k: np.asarray(v) for k, v in inputs.items()})



# revision 29
# speedup vs baseline: 1.0095x; 1.0095x over previous
"""Fused decoder-layer kernel for one TRN2 chip (8 NeuronCores).

Problem (B=2, S=2048, DIM=1024, H=16, DH=64, DFF=2048):
    h1 = MHA(q=de_x, k=de_x, v=de_x, mask)   (shared per-head weights Wq/Wk/Wv)
    h2 = MHA(q=en_x, k=en_x, v=h1,  None)
    y  = relu(h2 @ W1 + b1) @ W2 + b2

Sharding: core c = 4*b + g  (b = batch, g = head-group of 4 heads; g also
indexes the S/4 slice of rows this core runs the FFN on).

Layout strategy (all matmul operands pre-transposed so every contraction has
its reduction dim on SBUF partitions; all matmul inputs bf16, PSUM f32):
  - host passes x^T [DIM, S] per batch; projections produce q^T/k^T [DH, S]
    and v [S, DH] directly.
  - scores are computed transposed ([s2, s1]) so P^T feeds the PV matmul as
    stationary weights with K = s2 on partitions.
  - a ones-column appended to v makes the PV matmul also produce softmax row
    sums (flash-attention style); normalization happens on the [DH, S] output.
  - FFN runs column-transposed (ff1^T, y^T) so b1/b2 are per-partition ACT
    biases and no transposes are ever needed. Host un-transposes y^T.
Cross-core (v2 — collective-latency-optimized):
  - v2 = h1 @ Wv is computed as per-core PARTIAL products over the 256
    h1-features this core owns, then a 4-way ReduceScatter (add) within each
    batch group sums them and hands each core its own heads' 256 v-columns.
    Split into two s-halves so the first RS hides under attention-1 t=1.
  - h2^T is exchanged with four per-head 8-way AllToAlls, each fired as soon
    as that head's units finish so the wire time hides under the remaining
    attention compute. Foreign-batch rows are zeroed/folded via a tiny
    per-core 2-column mask; W1 is host-permuted so each folded head chunk
    contracts with a contiguous 128-row stationary slice, letting the FFN
    first layer accumulate incrementally per head.
  - PSUM: scores 2x[128,1024] + PV accumulator 1x[65,1024] + aux 2x[128,512]
    (projections / v2-partials / FFN) = exactly 8 banks. PSUM->SBUF copies
    run on the idle Pool (gpsimd) engine to keep DVE free for normalization.
"""

import math

import numpy as np
import ml_dtypes

import concourse.bass as bass
import concourse.bacc as bacc
import concourse.mybir as mybir
import concourse.tile as tile
import concourse.bass_utils as bass_utils

B, S, DIM, H = 2, 2048, 1024, 16
DH = DIM // H            # 64
DFF = 2048
NEG = -1.0e9
N_CORES = 8
G = 4                    # cores per batch group == head-groups == s-groups
HPC = H // G             # heads per core = 4
SL = S // G              # FFN rows per core = 512
KC = DIM // 128          # 8 k-chunks of the model dim
NB = S // 128            # 16 key blocks
NT = S // 1024           # 2 query tiles of 1024
BF16 = mybir.dt.bfloat16
F32 = mybir.dt.float32
AF = mybir.ActivationFunctionType

_CACHE: dict = {}


def _mask_plan(mask: np.ndarray):
    """Classify each [1024 x 128] (s1, s2) block: 'N' no-op, 'M' apply."""
    plan = []
    for t in range(NT):
        row = []
        for blk in range(NB):
            sub = mask[t * 1024:(t + 1) * 1024, blk * 128:(blk + 1) * 128]
            # NOTE: an 'S' (skip-block) fast path deadlocked on hardware;
            # fully-masked blocks share one all-NEG tile ('F', exp -> 0).
            row.append('N' if not sub.any() else ('F' if sub.all() else 'M'))
        plan.append(tuple(row))
    return tuple(plan)


def _build(plan):
    has_mask = any(c in 'MF' for row in plan for c in row)
    nc = bacc.Bacc("TRN2", target_bir_lowering=False, debug=False,
                   num_devices=N_CORES)

    de_xT = nc.dram_tensor("de_xT", [DIM, S], BF16, kind="ExternalInput")
    en_xT = nc.dram_tensor("en_xT", [DIM, S], BF16, kind="ExternalInput")
    wq = nc.dram_tensor("wq", [DIM, HPC * DH], BF16, kind="ExternalInput")
    wk = nc.dram_tensor("wk", [DIM, HPC * DH], BF16, kind="ExternalInput")
    wv = nc.dram_tensor("wv", [DIM, HPC * DH], BF16, kind="ExternalInput")
    wv2 = nc.dram_tensor("wv2", [HPC * DH, DIM], BF16, kind="ExternalInput")
    w1 = nc.dram_tensor("w1", [DIM, DFF], BF16, kind="ExternalInput")
    w2 = nc.dram_tensor("w2", [DFF, DIM], BF16, kind="ExternalInput")
    b1t = nc.dram_tensor("b1t", [128, DFF // 128], F32, kind="ExternalInput")
    b2t = nc.dram_tensor("b2t", [128, DIM // 128], F32, kind="ExternalInput")
    zmask = nc.dram_tensor("zmask", [128, 2], F32, kind="ExternalInput")
    maskT = None
    if has_mask:
        maskT = nc.dram_tensor("maskT", [S, S], BF16, kind="ExternalInput")
    yT = nc.dram_tensor("yT", [DIM, SL], F32, kind="ExternalOutput")

    with tile.TileContext(nc) as tc:
        _trace(nc, tc, plan, de_xT, en_xT, wq, wk, wv, wv2, w1, w2, b1t, b2t,
               zmask, maskT, yT)
    nc.compile()
    return nc, has_mask


def _trace(nc, tc, plan, de_xT, en_xT, wq, wk, wv, wv2, w1, w2, b1t, b2t,
           zmask, maskT, yT):
    # Unified allocation stack: everything (pools and single tiles) must be
    # released in strict LIFO order before TileContext exits.
    stack = nc._tile_stack = []   # [release_fn or None(done)]

    def _push(release_fn):
        ent = {"f": release_fn}
        stack.append(ent)
        def rel():
            assert ent["f"] is not None, "double release"
            ent["f"](); ent["f"] = None
        return rel

    def release_rest():
        for ent in reversed(stack):
            if ent["f"] is not None:
                ent["f"](); ent["f"] = None

    noplan = tuple(tuple('N' for _ in range(NB)) for _ in range(NT))

    # ---- pools ----------------------------------------------------------
    def pool(**kw):
        cm = tc.tile_pool(**kw)
        p = cm.__enter__()
        _push(lambda: cm.__exit__(None, None, None))
        return p

    def single(shape, dtype, name):
        t_, f_ = tc.tile(shape, dtype, name=name)
        return t_, _push(f_)

    has_mask = any(c in 'MF' for row in plan for c in row)
    ps_sc = pool(name="ps_sc", bufs=2, space="PSUM")     # [128,1024] scores/proj
    ps_hx = pool(name="ps_hx", bufs=1, space="PSUM")     # [65,1024] PV accum
    ps_aux = pool(name="ps_aux", bufs=2, space="PSUM")   # [128,512] v2p/FFN
    # pt depth = exp run-ahead across the RS-1 latency gap; the masked build
    # spends 64KB/partition on mask tiles so it gets a shallower pool.
    pt_pool = pool(name="pt", bufs=6 if has_mask else 15)
    rc_pool = pool(name="rc", bufs=1)
    bc_pool = pool(name="bc", bufs=1)
    # bufs=3: q1 pair-0/1 plus the filler-projected q2 pair-0 coexist
    q_pool = pool(name="qp", bufs=3)
    k_pool = pool(name="kp", bufs=3)
    v_pool = pool(name="vp", bufs=2)
    sh_pool = pool(name="shp", bufs=2)
    y_pool = pool(name="yp", bufs=1)
    fold_pool = pool(name="fold", bufs=4)
    dram = pool(name="dram", bufs=1, space="DRAM")

    # ---- persistent tiles (stack order: frees must pop LIFO) ------------
    wq_sb, _ = single([128, KC, HPC * DH], BF16, "wqsb")
    wk_sb, _ = single([128, KC, HPC * DH], BF16, "wksb")
    wv_sb, _ = single([128, KC, HPC * DH], BF16, "wvsb")
    wv2_sb, _ = single([128, 2, DIM], BF16, "wv2sb")
    # warm the ACT exp table while input DMAs stream (allocated below the
    # en/de stack so it is never popped before them)
    warm_sb, _ = single([128, 8], F32, "warm")
    nc.vector.memset(warm_sb[:], 0.0)
    nc.scalar.activation(warm_sb[:], warm_sb[:], AF.Exp)
    # en allocated BELOW de on the stack: de is freed first (after attn1),
    # en later (after attn2 q/k projections).
    en_sb, en_free = [], []
    for kc in range(KC):
        t_, f_ = single([128, S], BF16, f"en{kc}")
        en_sb.append(t_); en_free.append(f_)
    de_sb, de_free = [], []
    for kc in range(KC):
        t_, f_ = single([128, S], BF16, f"de{kc}")
        de_sb.append(t_); de_free.append(f_)
    # DMA trace order: attn1's operands first
    nc.sync.dma_start(wq_sb[:], wq.rearrange("(a p) c -> p a c", p=128))
    nc.sync.dma_start(wk_sb[:], wk.rearrange("(a p) c -> p a c", p=128))
    for kc in range(KC):
        nc.sync.dma_start(de_sb[kc][:], de_xT[kc * 128:(kc + 1) * 128, :])
    nc.sync.dma_start(wv_sb[:], wv.rearrange("(a p) c -> p a c", p=128))
    nc.sync.dma_start(wv2_sb[:], wv2.rearrange("(a p) c -> p a c", p=128))
    for kc in range(KC):
        nc.sync.dma_start(en_sb[kc][:], en_xT[kc * 128:(kc + 1) * 128, :])

    # collective bounce buffers.
    # v2 ReduceScatter, one per s-half: in = 4 chunks (head-groups) of
    # [1024 s, 256 e]; out = this core's summed [1024 s, 256 e].
    ccv_in = [dram.tile([G * 1024, HPC * DH], BF16, name=f"ccvi{t}")
              for t in range(NT)]
    ccv_out = [dram.tile([1024, HPC * DH], BF16, name=f"ccvo{t}")
               for t in range(NT)]
    # per-head h2 AllToAll: in = 8 chunks of [64 f, 512 s]; out = 8 blocks.
    cch_in = [dram.tile([N_CORES * DH, SL], BF16, name=f"cchi{h}")
              for h in range(HPC)]
    cch_out = [dram.tile([N_CORES * DH, SL], BF16, name=f"ccho{h}")
               for h in range(HPC)]

    # ---- helpers --------------------------------------------------------
    # Filler queue: small PE work items drained one per score-block inside
    # attention units. ACT paces attention (~1.1us/block) while PE only needs
    # ~0.85us, so ~1 extra matmul per block rides for free instead of a
    # projection/FFN block stalling the exp pipeline for 7-15us.
    filler = []

    def drain_filler(k):
        for _ in range(k):
            if not filler:
                return
            filler.pop(0)()

    def drain_all():
        while filler:
            filler.pop(0)()

    def project_qk_pair(x_sb, w_sb, pool, pair):
        """q^T (or k^T) for one head pair as a [128, S] bf16 tile."""
        qt = pool.tile([128, S], BF16, tag="qk", name=f"qk{pair}")
        for st in range(NT):
            ps = ps_sc.tile([128, 1024], F32, tag="ps", name="pjps")
            for kc in range(KC):
                for nn in (0, 512):
                    nc.tensor.matmul(
                        ps[:, nn:nn + 512],
                        w_sb[:, kc, pair * 128:(pair + 1) * 128],
                        x_sb[kc][:, st * 1024 + nn:st * 1024 + nn + 512],
                        start=(kc == 0), stop=(kc == KC - 1))
            nc.vector.tensor_copy(qt[:, st * 1024:(st + 1) * 1024], ps[:])
        return qt

    def project_qk_pair_f(x_sb, w_sb, pool, pair):
        """Filler variant: emits the projection as 512-col chunks of 2
        matmuls per item into the aux PSUM pool; returns the tile handle
        immediately (writes land as the filler drains)."""
        qt = pool.tile([128, S], BF16, tag="qk", name=f"qkf{pair}")
        for st in range(NT):
            for nn in (0, 512):
                state = {}
                for kc0 in range(0, KC, 2):
                    def item(st=st, nn=nn, kc0=kc0, state=state):
                        if kc0 == 0:
                            state["ps"] = ps_aux.tile(
                                [128, 512], F32, tag="aps", name="pjf")
                        for kc in (kc0, kc0 + 1):
                            nc.tensor.matmul(
                                state["ps"][:],
                                w_sb[:, kc, pair * 128:(pair + 1) * 128],
                                x_sb[kc][:, st * 1024 + nn:
                                          st * 1024 + nn + 512],
                                start=(kc == 0), stop=(kc == KC - 1))
                    filler.append(item)
                def fin(st=st, nn=nn, state=state):
                    nc.vector.tensor_copy(
                        qt[:, st * 1024 + nn:st * 1024 + nn + 512],
                        state["ps"][:])
                filler.append(fin)
        return qt

    def project_v(src_sb, v_all, blk_lo, blk_hi):
        """v for 4 heads + ones column into v_all[128, NB*HPC*65] (bf16)."""
        for blk in range(blk_lo, blk_hi):
            ps = ps_sc.tile([128, 1024], F32, tag="ps", name="vps")
            for kc in range(KC):
                nc.tensor.matmul(
                    ps[:, 0:HPC * DH],
                    src_sb[kc][:, blk * 128:(blk + 1) * 128],
                    wv_sb[:, kc, :],
                    start=(kc == 0), stop=(kc == KC - 1))
            for h in range(HPC):
                nc.vector.tensor_copy(
                    v_all[:, (blk * HPC + h) * 65:(blk * HPC + h) * 65 + 64],
                    ps[:, h * DH:(h + 1) * DH])

    def new_v_all():
        v_all = v_pool.tile([128, NB * HPC * 65], BF16, tag="v", name="vall")
        v3 = v_all[:].rearrange("p (b c) -> p b c", c=65)
        nc.vector.memset(v3[:, :, 64:65], 1.0)
        return v_all

    def attn_unit(q_pairs, k_pairs, v_all, aplan, mask_tiles_in, sh, h, t):
        """scores -> exp -> PV -> normalize for one (head, query-tile)."""
        pair, off = h // 2, (h % 2) * 64
        hext = ps_hx.tile([65, 1024], F32, name="hext")
        for blk in range(NB):
            sc = ps_sc.tile([128, 1024], F32, tag="ps", name="scps")
            for nn in (0, 512):
                nc.tensor.matmul(
                    sc[:, nn:nn + 512],
                    k_pairs[pair][off:off + 64, blk * 128:(blk + 1) * 128],
                    q_pairs[pair][off:off + 64,
                                  t * 1024 + nn:t * 1024 + nn + 512],
                    start=True, stop=True)
            if aplan[t][blk] == 'M':
                nc.vector.tensor_add(sc[:], sc[:], mask_tiles_in[(t, blk)][:])
            elif aplan[t][blk] == 'F':
                nc.vector.tensor_add(sc[:], sc[:], mask_tiles_in['F'][:])
            pt = pt_pool.tile([128, 1024], BF16, name="pt")
            nc.scalar.activation(pt[:], sc[:], AF.Exp)
            vs = v_all[:, (blk * HPC + h) * 65:(blk * HPC + h) * 65 + 65]
            for nn in (0, 512):
                nc.tensor.matmul(
                    hext[:, nn:nn + 512], vs, pt[:, nn:nn + 512],
                    start=(blk == 0), stop=(blk == NB - 1))
            drain_filler(2 if len(filler) > 28 else 1)
        recip = rc_pool.tile([1, 1024], F32, name="recip")
        nc.vector.reciprocal(recip[:], hext[64:65, :])
        rbc = bc_pool.tile([64, 1024], F32, name="rbc")
        nc.gpsimd.partition_broadcast(rbc[:], recip[0:1, :])
        nc.vector.tensor_mul(
            sh[pair][off:off + 64, t * 1024:(t + 1) * 1024],
            hext[0:64, :], rbc[:])

    def new_sh():
        return [sh_pool.tile([128, S], BF16, tag="sh", name=f"sh{p}")
                for p in range(2)]

    def v2_partial_half(h1sh, t):
        """Partial v2 (own 256 h1 features x all 1024 v-cols) for s-half t,
        DMA'd into the RS bounce buffer as 4 head-group chunks."""
        for sb in range(8):
            col = t * 1024 + sb * 128
            vp = fold_pool.tile([128, DIM], BF16, tag="v2p", name="v2p")
            for eh in (0, 512):
                ps = ps_aux.tile([128, 512], F32, tag="aps", name="v2ps")
                for pair in range(2):
                    nc.tensor.matmul(
                        ps[:],
                        h1sh[pair][:, col:col + 128],
                        wv2_sb[:, pair, eh:eh + 512],
                        start=(pair == 0), stop=(pair == 1))
                # DVE, not Pool: the q2/k2 projection copies ride the Pool
                # queue and must not stall behind these
                nc.vector.tensor_copy(vp[:, eh:eh + 512], ps[:])
            for g in range(G):
                nc.sync.dma_start(
                    ccv_in[t][g * 1024 + sb * 128:g * 1024 + (sb + 1) * 128, :],
                    vp[:, g * HPC * DH:(g + 1) * HPC * DH])
        nc.gpsimd.collective_compute(
            "ReduceScatter", mybir.AluOpType.add,
            replica_groups=[[0, 1, 2, 3], [4, 5, 6, 7]],
            ins=[ccv_in[t].opt()], outs=[ccv_out[t].opt()])

    def v2_fill_half(v_all, t):
        """DMA the reduce-scattered v2 s-half into v_all's per-head slots."""
        v3 = v_all[:].rearrange("p (b h c) -> p b h c", h=HPC, c=65)
        for sb in range(8):
            blk = t * 8 + sb
            nc.sync.dma_start(
                v3[:, blk, :, 0:64],
                ccv_out[t][sb * 128:(sb + 1) * 128, :]
                .rearrange("p (h c) -> p h c", c=64))

    # ---- attention 1 (self-attn on de_x, mask) --------------------------
    # pair-0 q/k and v project first so the exp pipeline (ACT) starts as
    # early as possible; pair-1 projections slot in behind the first units.
    q1 = [None, None]
    k1 = [None, None]
    q1[0] = project_qk_pair(de_sb, wq_sb, q_pool, 0)
    k1[0] = project_qk_pair(de_sb, wk_sb, k_pool, 0)
    v1 = new_v_all()
    project_v(de_sb, v1, 0, NB)
    mask_tiles, mask_free = {}, []
    if any(c == 'F' for row in plan for c in row):
        ft, ff = single([128, 1024], BF16, "mkF")
        nc.vector.memset(ft[:], NEG)
        mask_tiles['F'] = ft
        mask_free.append(ff)
    for t in range(NT):
        for blk in range(NB):
            if plan[t][blk] == 'M':
                mt, fm = single([128, 1024], BF16, f"mk{t}_{blk}")
                nc.sync.dma_start(
                    mt[:], maskT[blk * 128:(blk + 1) * 128,
                                 t * 1024:(t + 1) * 1024])
                mask_tiles[(t, blk)] = mt
                mask_free.append(fm)
    # t-major so each s-half of the partial-v2 ReduceScatter fires while the
    # other half's attention units are still computing.
    h1sh = new_sh()
    v2 = new_v_all()
    q2 = [None, None]
    k2 = [None, None]
    for t in range(NT):
        for h in range(HPC):
            if t == 0 and h == 0:
                # pair-1 projections ride t0's ACT slack via the filler
                q1[1] = project_qk_pair_f(de_sb, wq_sb, q_pool, 1)
                k1[1] = project_qk_pair_f(de_sb, wk_sb, k_pool, 1)
            if t == 0 and h == 2:
                drain_all()   # pair-1 q/k must be fully written before use
            # attn2 pair-0 projections ride t1's slack so exp2 can start
            # the moment attention 1 drains (hiding RS-1)
            if t == 1 and h == 0:
                q2[0] = project_qk_pair_f(en_sb, wq_sb, q_pool, 0)
                k2[0] = project_qk_pair_f(en_sb, wk_sb, k_pool, 0)
            attn_unit(q1, k1, v1, plan, mask_tiles, h1sh, h, t)
        drain_all()
        if t == 0:
            v2_partial_half(h1sh, 0)
    for f in reversed(mask_free):
        f()
    for f in reversed(de_free):
        f()

    # ---- attention 2 (q,k from en_x; v from reduce-scattered h1@Wv) -----
    # Order on PE after the last attn1 unit: t=1 v2 partials (starts RS-1
    # early); the pair-1 projections ride attn2-h0's units via the filler.
    v2_partial_half(h1sh, 1)
    v2_fill_half(v2, 0)
    # pair-1 projections emitted directly: their PE time runs parallel to
    # the RS-1 wire wait, and deferring them past the en frees would break
    # the allocator's happens-before on the reused en space.
    q2[1] = project_qk_pair(en_sb, wq_sb, q_pool, 1)
    k2[1] = project_qk_pair(en_sb, wk_sb, k_pool, 1)
    v2_fill_half(v2, 1)
    for f in reversed(en_free):
        f()

    # FFN weights prefetch during attention 2 (w1 is host-permuted so that
    # per-head folded A2A chunks hit contiguous 128-row stationary slices)
    w1_sb, w1_free = [], []
    for kc in range(KC):
        t_, f_ = single([128, DFF], BF16, f"w1_{kc}")
        nc.sync.dma_start(t_[:], w1[kc * 128:(kc + 1) * 128, :])
        w1_sb.append(t_); w1_free.append(f_)
    b1_sb, _ = single([128, DFF // 128], F32, "b1sb")
    b2_sb, _ = single([128, DIM // 128], F32, "b2sb")
    zm_sb, _ = single([128, 2], F32, "zmsb")
    nc.sync.dma_start(b1_sb[:], b1t[:])
    nc.sync.dma_start(b2_sb[:], b2t[:])
    nc.sync.dma_start(zm_sb[:], zmask[:])
    w2_sb, w2_free = [], []
    for dc in range(DFF // 128):
        t_, f_ = single([128, DIM], BF16, f"w2_{dc}")
        w2_sb.append(t_); w2_free.append(f_)
    ff1_sb, ff1_free = [], []
    for dffb in range(DFF // 128):
        t_, f_ = single([128, SL], BF16, f"ff1_{dffb}")
        ff1_sb.append(t_); ff1_free.append(f_)

    def ffn_fold(h):
        """Fold head h's A2A arrival (zmask zeroes the foreign-batch copy).
        Recv DMAs ride the Pool queue so they never head-of-line-block the
        SP queue behind a later head's A2A input DMAs."""
        fold = []
        for i in range(2):
            lo = fold_pool.tile([128, SL], BF16, tag="fl", name=f"flo{h}_{i}")
            hi = fold_pool.tile([128, SL], BF16, tag="fh", name=f"fhi{h}_{i}")
            nc.gpsimd.dma_start(lo[:], cch_out[h][i * 128:(i + 1) * 128, :])
            nc.gpsimd.dma_start(
                hi[:], cch_out[h][256 + i * 128:256 + (i + 1) * 128, :])
            nc.vector.tensor_scalar_mul(lo[:], lo[:], zm_sb[:, 0:1])
            nc.vector.tensor_scalar_mul(hi[:], hi[:], zm_sb[:, 1:2])
            nc.vector.tensor_add(lo[:], lo[:], hi[:])
            fold.append(lo)
        return fold

    def ffn_w1(h, tail):
        """Head h's W1 contribution, accumulated in-place in ff1_sb.
        tail=False emits via the filler (safe: only queued two heads after
        the A2A fired, so the data is long since landed)."""
        fold = ffn_fold(h)
        for dffb in range(DFF // 128):
            state = {}
            def it_mm(h=h, dffb=dffb, fold=fold, state=state):
                ps = ps_aux.tile([128, 512], F32, tag="aps", name="f1ps")
                state["ps"] = ps
                for i in range(2):
                    nc.tensor.matmul(
                        ps[:, 0:SL],
                        w1_sb[2 * h + i][:, dffb * 128:(dffb + 1) * 128],
                        fold[i][:],
                        start=(i == 0), stop=(i == 1))
            def it_acc(h=h, dffb=dffb, state=state):
                ps = state["ps"]
                if h == 0:
                    nc.vector.tensor_copy(ff1_sb[dffb][:], ps[:, 0:SL])
                elif h < HPC - 1:
                    nc.vector.tensor_add(ff1_sb[dffb][:], ff1_sb[dffb][:],
                                         ps[:, 0:SL])
                else:
                    nc.vector.tensor_add(ps[:, 0:SL], ps[:, 0:SL],
                                         ff1_sb[dffb][:])
                    nc.scalar.activation(ff1_sb[dffb][:], ps[:, 0:SL],
                                         AF.Relu,
                                         bias=b1_sb[:, dffb:dffb + 1])
            if tail:
                it_mm(); it_acc()
            else:
                filler.append(it_mm); filler.append(it_acc)

    # h-major so each head's A2A fires while later heads are still
    # computing; head h-2's fold+W1 pass rides the filler through head h's
    # units (two heads back => its A2A has long completed).
    h2sh = new_sh()
    for h in range(HPC):
        if h >= 2:
            ffn_w1(h - 2, tail=False)
        for t in range(NT):
            attn_unit(q2, k2, v2, noplan, {}, h2sh, h, t)
        pair, off = h // 2, (h % 2) * 64
        for j in range(N_CORES):
            nc.sync.dma_start(
                cch_in[h][j * DH:(j + 1) * DH, :],
                h2sh[pair][off:off + 64, (j % G) * SL:(j % G + 1) * SL])
        nc.gpsimd.collective_compute(
            "AllToAll", mybir.AluOpType.bypass,
            replica_groups=[[0, 1, 2, 3, 4, 5, 6, 7]],
            ins=[cch_in[h].opt()], outs=[cch_out[h].opt()])
        if h == 0:
            # w2 prefetch now: late enough for SBUF headroom, early enough
            # not to block the tail
            for dc in range(DFF // 128):
                nc.sync.dma_start(w2_sb[dc][:], w2[dc * 128:(dc + 1) * 128, :])

    # ---- FFN tail on own S/4 rows --------------------------------------
    drain_all()
    ffn_w1(HPC - 2, tail=True)
    ffn_w1(HPC - 1, tail=True)
    for dimb in range(DIM // 128):
        ps = ps_aux.tile([128, 512], F32, tag="aps", name="yps")
        for dc in range(DFF // 128):
            nc.tensor.matmul(
                ps[:, 0:SL],
                w2_sb[dc][:, dimb * 128:(dimb + 1) * 128],
                ff1_sb[dc][:],
                start=(dc == 0), stop=(dc == DFF // 128 - 1))
        ysb = y_pool.tile([128, SL], F32, tag="y", name="ysb")
        nc.vector.tensor_scalar_add(ysb[:], ps[:, 0:SL],
                                    b2_sb[:, dimb:dimb + 1])
        nc.sync.dma_start(yT[dimb * 128:(dimb + 1) * 128, :], ysb[:])

    release_rest()


def _prep_inputs(de_x, en_x, mask, Wq, Wk, Wv, W1, b1, W2, b2, has_mask):
    bf = ml_dtypes.bfloat16
    scale = 1.0 / math.sqrt(DH)
    in_maps = []
    deT = [np.ascontiguousarray(de_x[b].T).astype(bf) for b in range(B)]
    enT = [np.ascontiguousarray(en_x[b].T).astype(bf) for b in range(B)]
    # W1 rows permuted so that per-head A2A fold chunks are contiguous:
    # w1p[h*256 + j*64 + r] = W1[(j*4 + h)*64 + r]
    perm = np.empty(DIM, np.int64)
    for h in range(HPC):
        for j in range(G):
            perm[h * 256 + j * 64:h * 256 + (j + 1) * 64] = \
                np.arange((j * G + h) * 64, (j * G + h) * 64 + 64)
    w1p = np.ascontiguousarray(W1[perm]).astype(bf)
    w2b = W2.astype(bf)
    b1t = np.ascontiguousarray(b1.reshape(DFF // 128, 128).T).astype(np.float32)
    b2t = np.ascontiguousarray(b2.reshape(DIM // 128, 128).T).astype(np.float32)
    wv_flat = np.ascontiguousarray(
        np.transpose(Wv, (1, 0, 2)).reshape(DIM, H * DH)).astype(bf)
    mT = None
    if has_mask:
        mT = np.ascontiguousarray(mask.T * np.float32(NEG)).astype(bf)
    for c in range(N_CORES):
        b, g = divmod(c, G)
        hs = slice(g * HPC, (g + 1) * HPC)
        m = {
            "de_xT": deT[b],
            "en_xT": enT[b],
            "wq": np.ascontiguousarray(
                np.transpose(Wq[hs] * scale, (1, 0, 2)).reshape(DIM, HPC * DH)
            ).astype(bf),
            "wk": np.ascontiguousarray(
                np.transpose(Wk[hs], (1, 0, 2)).reshape(DIM, HPC * DH)).astype(bf),
            "wv": np.ascontiguousarray(
                np.transpose(Wv[hs], (1, 0, 2)).reshape(DIM, HPC * DH)).astype(bf),
            # partial-v2 weights: rows = this core's 256 h1-features,
            # cols = all 16 heads' v outputs
            "wv2": np.ascontiguousarray(
                wv_flat[g * HPC * DH:(g + 1) * HPC * DH, :]),
            "w1": w1p, "w2": w2b, "b1t": b1t, "b2t": b2t,
        }
        zm = np.zeros((128, 2), np.float32)
        zm[:, 0 if b == 0 else 1] = 1.0
        m["zmask"] = zm
        if has_mask:
            m["maskT"] = mT
        in_maps.append(m)
    return in_maps


def get_program(mask):
    plan = _mask_plan(np.asarray(mask))
    if plan not in _CACHE:
        _CACHE[plan] = _build(plan)
    return _CACHE[plan]


_RUNNERS: dict = {}


def _fast_runner(nc):
    """Build (once) a cached jitted SPMD executor for this program.

    run_bass_kernel_spmd re-creates and re-traces its jax.jit closure on
    every call; caching the jitted shard_map shaves seconds of dispatch
    overhead off warm calls. Mirrors bass2jax.run_bass_via_pjrt.
    """
    import jax
    from jax.sharding import Mesh, PartitionSpec
    try:
        from jax.experimental.shard_map import shard_map
    except ImportError:
        from jax.shard_map import shard_map
    import concourse.mybir as _mb
    from concourse import bass2jax as b2j

    b2j.install_neuronx_cc_hook()
    partition_name = (nc.partition_id_tensor.name
                      if nc.partition_id_tensor else None)
    in_names, out_names, out_avals = [], [], []
    for alloc in nc.m.functions[0].allocations:
        if not isinstance(alloc, _mb.MemoryLocationSet):
            continue
        name = alloc.memorylocations[0].name
        if alloc.kind == "ExternalInput":
            if name != partition_name:
                in_names.append(name)
        elif alloc.kind == "ExternalOutput":
            out_names.append(name)
            out_avals.append(jax.core.ShapedArray(
                tuple(alloc.tensor_shape), _mb.dt.np(alloc.dtype)))
    n_params = len(in_names)
    n_outs = len(out_avals)
    all_names = in_names + out_names + ([partition_name] if partition_name else [])
    donate = tuple(range(n_params, n_params + n_outs))

    def _body(*args):
        operands = list(args)
        if partition_name is not None:
            operands.append(b2j.partition_id_tensor())
        return tuple(b2j._bass_exec_p.bind(
            *operands,
            out_avals=tuple(out_avals),
            in_names=tuple(all_names),
            out_names=tuple(out_names),
            lowering_input_output_aliases=(),
            sim_require_finite=True,
            sim_require_nnan=True,
            nc=nc,
        ))

    devices = jax.devices()[:N_CORES]
    mesh = Mesh(np.asarray(devices), ("core",))
    in_specs = (PartitionSpec("core"),) * (n_params + n_outs)
    out_specs = (PartitionSpec("core"),) * n_outs
    sharded = jax.jit(
        shard_map(_body, mesh=mesh, in_specs=in_specs, out_specs=out_specs,
                  check_rep=False),
        donate_argnums=donate, keep_unused=True)

    def runner(in_maps):
        concat_in = [np.concatenate([in_maps[c][nm] for c in range(N_CORES)],
                                    axis=0) for nm in in_names]
        zeros = [np.zeros((N_CORES * a.shape[0], *a.shape[1:]), a.dtype)
                 for a in out_avals]
        out_arrs = sharded(*concat_in, *zeros)
        return [
            {nm: np.asarray(out_arrs[i]).reshape(N_CORES, *out_avals[i].shape)[c]
             for i, nm in enumerate(out_names)}
            for c in range(N_CORES)
        ]

    return runner


def run(inputs, want_results=False, **run_kwargs):
    nc, has_mask = get_program(inputs["mask"])
    in_maps = _prep_inputs(
        inputs["de_x"], inputs["en_x"], inputs["mask"],
        inputs["Wq"], inputs["Wk"], inputs["Wv"],
        inputs["W1"], inputs["b1"], inputs["W2"], inputs["b2"], has_mask)
    results = None
    res = None
    if not run_kwargs:
        try:
            key = id(nc)
            if key not in _RUNNERS:
                _RUNNERS[key] = _fast_runner(nc)
            results = _RUNNERS[key](in_maps)
        except Exception:
            results = None
    if results is None:
        res = bass_utils.run_bass_kernel_spmd(
            nc, in_maps, core_ids=list(range(N_CORES)), **run_kwargs)
        results = res.results
    y = np.empty((B, S, DIM), np.float32)
    for c in range(N_CORES):
        b, g = divmod(c, G)
        y[b, g * SL:(g + 1) * SL, :] = results[c]["yT"].T
    return (y, res) if want_results else y


def kernel(**inputs) -> np.ndarray:
    return run({k: np.asarray(v) for k, v in inputs.items()})


# revision 30
# speedup vs baseline: 1.0144x; 1.0049x over previous
"""Fused decoder-layer kernel for one TRN2 chip (8 NeuronCores).

Problem (B=2, S=2048, DIM=1024, H=16, DH=64, DFF=2048):
    h1 = MHA(q=de_x, k=de_x, v=de_x, mask)   (shared per-head weights Wq/Wk/Wv)
    h2 = MHA(q=en_x, k=en_x, v=h1,  None)
    y  = relu(h2 @ W1 + b1) @ W2 + b2

Sharding: core c = 4*b + g  (b = batch, g = head-group of 4 heads; g also
indexes the S/4 slice of rows this core runs the FFN on).

Layout strategy (all matmul operands pre-transposed so every contraction has
its reduction dim on SBUF partitions; all matmul inputs bf16, PSUM f32):
  - host passes x^T [DIM, S] per batch; projections produce q^T/k^T [DH, S]
    and v [S, DH] directly.
  - scores are computed transposed ([s2, s1]) so P^T feeds the PV matmul as
    stationary weights with K = s2 on partitions.
  - a ones-column appended to v makes the PV matmul also produce softmax row
    sums (flash-attention style); normalization happens on the [DH, S] output.
  - FFN runs column-transposed (ff1^T, y^T) so b1/b2 are per-partition ACT
    biases and no transposes are ever needed. Host un-transposes y^T.
Cross-core (v2 — collective-latency-optimized):
  - v2 = h1 @ Wv is computed as per-core PARTIAL products over the 256
    h1-features this core owns, then a 4-way ReduceScatter (add) within each
    batch group sums them and hands each core its own heads' 256 v-columns.
    Split into two s-halves so the first RS hides under attention-1 t=1.
  - h2^T is exchanged with four per-head 8-way AllToAlls, each fired as soon
    as that head's units finish so the wire time hides under the remaining
    attention compute. Foreign-batch rows are zeroed/folded via a tiny
    per-core 2-column mask; W1 is host-permuted so each folded head chunk
    contracts with a contiguous 128-row stationary slice, letting the FFN
    first layer accumulate incrementally per head.
  - PSUM: scores 2x[128,1024] + PV accumulator 1x[65,1024] + aux 2x[128,512]
    (projections / v2-partials / FFN) = exactly 8 banks. PSUM->SBUF copies
    run on the idle Pool (gpsimd) engine to keep DVE free for normalization.
"""

import math

import numpy as np
import ml_dtypes

import concourse.bass as bass
import concourse.bacc as bacc
import concourse.mybir as mybir
import concourse.tile as tile
import concourse.bass_utils as bass_utils

B, S, DIM, H = 2, 2048, 1024, 16
DH = DIM // H            # 64
DFF = 2048
NEG = -1.0e9
N_CORES = 8
G = 4                    # cores per batch group == head-groups == s-groups
HPC = H // G             # heads per core = 4
SL = S // G              # FFN rows per core = 512
KC = DIM // 128          # 8 k-chunks of the model dim
NB = S // 128            # 16 key blocks
NT = S // 1024           # 2 query tiles of 1024
BF16 = mybir.dt.bfloat16
F32 = mybir.dt.float32
AF = mybir.ActivationFunctionType

_CACHE: dict = {}


def _mask_plan(mask: np.ndarray):
    """Classify each [1024 x 128] (s1, s2) block: 'N' no-op, 'M' apply."""
    plan = []
    for t in range(NT):
        row = []
        for blk in range(NB):
            sub = mask[t * 1024:(t + 1) * 1024, blk * 128:(blk + 1) * 128]
            # NOTE: an 'S' (skip-block) fast path deadlocked on hardware;
            # fully-masked blocks share one all-NEG tile ('F', exp -> 0).
            row.append('N' if not sub.any() else ('F' if sub.all() else 'M'))
        plan.append(tuple(row))
    return tuple(plan)


def _build(plan):
    has_mask = any(c in 'MF' for row in plan for c in row)
    nc = bacc.Bacc("TRN2", target_bir_lowering=False, debug=False,
                   num_devices=N_CORES)

    de_xT = nc.dram_tensor("de_xT", [DIM, S], BF16, kind="ExternalInput")
    en_xT = nc.dram_tensor("en_xT", [DIM, S], BF16, kind="ExternalInput")
    wq = nc.dram_tensor("wq", [DIM, HPC * DH], BF16, kind="ExternalInput")
    wk = nc.dram_tensor("wk", [DIM, HPC * DH], BF16, kind="ExternalInput")
    wv = nc.dram_tensor("wv", [DIM, HPC * DH], BF16, kind="ExternalInput")
    wv2 = nc.dram_tensor("wv2", [HPC * DH, DIM], BF16, kind="ExternalInput")
    w1 = nc.dram_tensor("w1", [DIM, DFF], BF16, kind="ExternalInput")
    w2 = nc.dram_tensor("w2", [DFF, DIM], BF16, kind="ExternalInput")
    b1t = nc.dram_tensor("b1t", [128, DFF // 128], F32, kind="ExternalInput")
    b2t = nc.dram_tensor("b2t", [128, DIM // 128], F32, kind="ExternalInput")
    zmask = nc.dram_tensor("zmask", [128, 2], F32, kind="ExternalInput")
    maskT = None
    if has_mask:
        maskT = nc.dram_tensor("maskT", [S, S], BF16, kind="ExternalInput")
    yT = nc.dram_tensor("yT", [DIM, SL], F32, kind="ExternalOutput")

    with tile.TileContext(nc) as tc:
        _trace(nc, tc, plan, de_xT, en_xT, wq, wk, wv, wv2, w1, w2, b1t, b2t,
               zmask, maskT, yT)
    nc.compile()
    return nc, has_mask


def _trace(nc, tc, plan, de_xT, en_xT, wq, wk, wv, wv2, w1, w2, b1t, b2t,
           zmask, maskT, yT):
    # Unified allocation stack: everything (pools and single tiles) must be
    # released in strict LIFO order before TileContext exits.
    stack = nc._tile_stack = []   # [release_fn or None(done)]

    def _push(release_fn):
        ent = {"f": release_fn}
        stack.append(ent)
        def rel():
            assert ent["f"] is not None, "double release"
            ent["f"](); ent["f"] = None
        return rel

    def release_rest():
        for ent in reversed(stack):
            if ent["f"] is not None:
                ent["f"](); ent["f"] = None

    noplan = tuple(tuple('N' for _ in range(NB)) for _ in range(NT))

    # ---- pools ----------------------------------------------------------
    def pool(**kw):
        cm = tc.tile_pool(**kw)
        p = cm.__enter__()
        _push(lambda: cm.__exit__(None, None, None))
        return p

    def single(shape, dtype, name):
        t_, f_ = tc.tile(shape, dtype, name=name)
        return t_, _push(f_)

    has_mask = any(c in 'MF' for row in plan for c in row)
    ps_sc = pool(name="ps_sc", bufs=2, space="PSUM")     # [128,1024] scores/proj
    ps_hx = pool(name="ps_hx", bufs=1, space="PSUM")     # [65,1024] PV accum
    ps_aux = pool(name="ps_aux", bufs=2, space="PSUM")   # [128,512] v2p/FFN
    # pt depth = exp run-ahead across the RS-1 latency gap; the masked build
    # spends 64KB/partition on mask tiles so it gets a shallower pool.
    pt_pool = pool(name="pt", bufs=6 if has_mask else 15)
    rc_pool = pool(name="rc", bufs=1)
    bc_pool = pool(name="bc", bufs=1)
    # bufs=3: q1 pair-0/1 plus the filler-projected q2 pair-0 coexist
    q_pool = pool(name="qp", bufs=3)
    k_pool = pool(name="kp", bufs=3)
    v_pool = pool(name="vp", bufs=2)
    sh_pool = pool(name="shp", bufs=2)
    y_pool = pool(name="yp", bufs=1)
    fold_pool = pool(name="fold", bufs=4)
    dram = pool(name="dram", bufs=1, space="DRAM")

    # ---- persistent tiles (stack order: frees must pop LIFO) ------------
    wq_sb, _ = single([128, KC, HPC * DH], BF16, "wqsb")
    wk_sb, _ = single([128, KC, HPC * DH], BF16, "wksb")
    wv_sb, _ = single([128, KC, HPC * DH], BF16, "wvsb")
    wv2_sb, _ = single([128, 2, DIM], BF16, "wv2sb")
    # warm the ACT exp table while input DMAs stream (allocated below the
    # en/de stack so it is never popped before them)
    warm_sb, _ = single([128, 8], F32, "warm")
    nc.vector.memset(warm_sb[:], 0.0)
    nc.scalar.activation(warm_sb[:], warm_sb[:], AF.Exp)
    # en allocated BELOW de on the stack: de is freed first (after attn1),
    # en later (after attn2 q/k projections).
    en_sb, en_free = [], []
    for kc in range(KC):
        t_, f_ = single([128, S], BF16, f"en{kc}")
        en_sb.append(t_); en_free.append(f_)
    de_sb, de_free = [], []
    for kc in range(KC):
        t_, f_ = single([128, S], BF16, f"de{kc}")
        de_sb.append(t_); de_free.append(f_)
    # DMA trace order: attn1's operands first
    nc.sync.dma_start(wq_sb[:], wq.rearrange("(a p) c -> p a c", p=128))
    nc.sync.dma_start(wk_sb[:], wk.rearrange("(a p) c -> p a c", p=128))
    for kc in range(KC):
        nc.sync.dma_start(de_sb[kc][:], de_xT[kc * 128:(kc + 1) * 128, :])
    nc.sync.dma_start(wv_sb[:], wv.rearrange("(a p) c -> p a c", p=128))
    nc.sync.dma_start(wv2_sb[:], wv2.rearrange("(a p) c -> p a c", p=128))
    for kc in range(KC):
        nc.sync.dma_start(en_sb[kc][:], en_xT[kc * 128:(kc + 1) * 128, :])

    # collective bounce buffers.
    # v2 ReduceScatter, one per s-half: in = 4 chunks (head-groups) of
    # [1024 s, 256 e]; out = this core's summed [1024 s, 256 e].
    ccv_in = [dram.tile([G * 1024, HPC * DH], BF16, name=f"ccvi{t}")
              for t in range(NT)]
    ccv_out = [dram.tile([1024, HPC * DH], BF16, name=f"ccvo{t}")
               for t in range(NT)]
    # per-head h2 AllToAll: in = 8 chunks of [64 f, 512 s]; out = 8 blocks.
    cch_in = [dram.tile([N_CORES * DH, SL], BF16, name=f"cchi{h}")
              for h in range(HPC)]
    cch_out = [dram.tile([N_CORES * DH, SL], BF16, name=f"ccho{h}")
               for h in range(HPC)]

    # ---- helpers --------------------------------------------------------
    # Filler queue: small PE work items drained one per score-block inside
    # attention units. ACT paces attention (~1.1us/block) while PE only needs
    # ~0.85us, so ~1 extra matmul per block rides for free instead of a
    # projection/FFN block stalling the exp pipeline for 7-15us.
    filler = []

    def drain_filler(k):
        for _ in range(k):
            if not filler:
                return
            filler.pop(0)()

    def drain_all():
        while filler:
            filler.pop(0)()

    def project_qk_pair(x_sb, w_sb, pool, pair):
        """q^T (or k^T) for one head pair as a [128, S] bf16 tile."""
        qt = pool.tile([128, S], BF16, tag="qk", name=f"qk{pair}")
        for st in range(NT):
            ps = ps_sc.tile([128, 1024], F32, tag="ps", name="pjps")
            for kc in range(KC):
                for nn in (0, 512):
                    nc.tensor.matmul(
                        ps[:, nn:nn + 512],
                        w_sb[:, kc, pair * 128:(pair + 1) * 128],
                        x_sb[kc][:, st * 1024 + nn:st * 1024 + nn + 512],
                        start=(kc == 0), stop=(kc == KC - 1))
            nc.vector.tensor_copy(qt[:, st * 1024:(st + 1) * 1024], ps[:])
        return qt

    def project_qk_pair_f(x_sb, w_sb, pool, pair):
        """Filler variant: emits the projection as 512-col chunks of 2
        matmuls per item into the aux PSUM pool; returns the tile handle
        immediately (writes land as the filler drains)."""
        qt = pool.tile([128, S], BF16, tag="qk", name=f"qkf{pair}")
        for st in range(NT):
            for nn in (0, 512):
                state = {}
                for kc0 in range(0, KC, 2):
                    def item(st=st, nn=nn, kc0=kc0, state=state):
                        if kc0 == 0:
                            state["ps"] = ps_aux.tile(
                                [128, 512], F32, tag="aps", name="pjf")
                        for kc in (kc0, kc0 + 1):
                            nc.tensor.matmul(
                                state["ps"][:],
                                w_sb[:, kc, pair * 128:(pair + 1) * 128],
                                x_sb[kc][:, st * 1024 + nn:
                                          st * 1024 + nn + 512],
                                start=(kc == 0), stop=(kc == KC - 1))
                    filler.append(item)
                def fin(st=st, nn=nn, state=state):
                    nc.vector.tensor_copy(
                        qt[:, st * 1024 + nn:st * 1024 + nn + 512],
                        state["ps"][:])
                filler.append(fin)
        return qt

    def project_v(src_sb, v_all, blk_lo, blk_hi):
        """v for 4 heads + ones column into v_all[128, NB*HPC*65] (bf16)."""
        for blk in range(blk_lo, blk_hi):
            ps = ps_sc.tile([128, 1024], F32, tag="ps", name="vps")
            for kc in range(KC):
                nc.tensor.matmul(
                    ps[:, 0:HPC * DH],
                    src_sb[kc][:, blk * 128:(blk + 1) * 128],
                    wv_sb[:, kc, :],
                    start=(kc == 0), stop=(kc == KC - 1))
            for h in range(HPC):
                nc.vector.tensor_copy(
                    v_all[:, (blk * HPC + h) * 65:(blk * HPC + h) * 65 + 64],
                    ps[:, h * DH:(h + 1) * DH])

    def new_v_all():
        v_all = v_pool.tile([128, NB * HPC * 65], BF16, tag="v", name="vall")
        v3 = v_all[:].rearrange("p (b c) -> p b c", c=65)
        nc.vector.memset(v3[:, :, 64:65], 1.0)
        return v_all

    def attn_unit(q_pairs, k_pairs, v_all, aplan, mask_tiles_in, sh, h, t):
        """scores -> exp -> PV -> normalize for one (head, query-tile)."""
        pair, off = h // 2, (h % 2) * 64
        hext = ps_hx.tile([65, 1024], F32, name="hext")
        for blk in range(NB):
            sc = ps_sc.tile([128, 1024], F32, tag="ps", name="scps")
            for nn in (0, 512):
                nc.tensor.matmul(
                    sc[:, nn:nn + 512],
                    k_pairs[pair][off:off + 64, blk * 128:(blk + 1) * 128],
                    q_pairs[pair][off:off + 64,
                                  t * 1024 + nn:t * 1024 + nn + 512],
                    start=True, stop=True)
            if aplan[t][blk] == 'M':
                nc.vector.tensor_add(sc[:], sc[:], mask_tiles_in[(t, blk)][:])
            elif aplan[t][blk] == 'F':
                nc.vector.tensor_add(sc[:], sc[:], mask_tiles_in['F'][:])
            pt = pt_pool.tile([128, 1024], BF16, name="pt")
            nc.scalar.activation(pt[:], sc[:], AF.Exp)
            vs = v_all[:, (blk * HPC + h) * 65:(blk * HPC + h) * 65 + 65]
            for nn in (0, 512):
                nc.tensor.matmul(
                    hext[:, nn:nn + 512], vs, pt[:, nn:nn + 512],
                    start=(blk == 0), stop=(blk == NB - 1))
            drain_filler(2 if len(filler) > 28 else 1)
        recip = rc_pool.tile([1, 1024], F32, name="recip")
        nc.vector.reciprocal(recip[:], hext[64:65, :])
        rbc = bc_pool.tile([64, 1024], F32, name="rbc")
        nc.gpsimd.partition_broadcast(rbc[:], recip[0:1, :])
        nc.vector.tensor_mul(
            sh[pair][off:off + 64, t * 1024:(t + 1) * 1024],
            hext[0:64, :], rbc[:])

    def new_sh():
        return [sh_pool.tile([128, S], BF16, tag="sh", name=f"sh{p}")
                for p in range(2)]

    def v2_partial_half(h1sh, t):
        """Partial v2 (own 256 h1 features x all 1024 v-cols) for s-half t,
        DMA'd into the RS bounce buffer as 4 head-group chunks."""
        for sb in range(8):
            col = t * 1024 + sb * 128
            vp = fold_pool.tile([128, DIM], BF16, tag="v2p", name="v2p")
            for eh in (0, 512):
                ps = ps_aux.tile([128, 512], F32, tag="aps", name="v2ps")
                for pair in range(2):
                    nc.tensor.matmul(
                        ps[:],
                        h1sh[pair][:, col:col + 128],
                        wv2_sb[:, pair, eh:eh + 512],
                        start=(pair == 0), stop=(pair == 1))
                # DVE, not Pool: the q2/k2 projection copies ride the Pool
                # queue and must not stall behind these
                nc.vector.tensor_copy(vp[:, eh:eh + 512], ps[:])
            for g in range(G):
                nc.sync.dma_start(
                    ccv_in[t][g * 1024 + sb * 128:g * 1024 + (sb + 1) * 128, :],
                    vp[:, g * HPC * DH:(g + 1) * HPC * DH])
        nc.gpsimd.collective_compute(
            "ReduceScatter", mybir.AluOpType.add,
            replica_groups=[[0, 1, 2, 3], [4, 5, 6, 7]],
            ins=[ccv_in[t].opt()], outs=[ccv_out[t].opt()])

    def v2_fill_half(v_all, t):
        """DMA the reduce-scattered v2 s-half into v_all's per-head slots."""
        v3 = v_all[:].rearrange("p (b h c) -> p b h c", h=HPC, c=65)
        for sb in range(8):
            blk = t * 8 + sb
            nc.sync.dma_start(
                v3[:, blk, :, 0:64],
                ccv_out[t][sb * 128:(sb + 1) * 128, :]
                .rearrange("p (h c) -> p h c", c=64))

    # ---- attention 1 (self-attn on de_x, mask) --------------------------
    # pair-0 q/k and v project first so the exp pipeline (ACT) starts as
    # early as possible; pair-1 projections slot in behind the first units.
    q1 = [None, None]
    k1 = [None, None]
    q1[0] = project_qk_pair(de_sb, wq_sb, q_pool, 0)
    k1[0] = project_qk_pair(de_sb, wk_sb, k_pool, 0)
    v1 = new_v_all()
    project_v(de_sb, v1, 0, NB)
    mask_tiles, mask_free = {}, []
    if any(c == 'F' for row in plan for c in row):
        ft, ff = single([128, 1024], BF16, "mkF")
        nc.vector.memset(ft[:], NEG)
        mask_tiles['F'] = ft
        mask_free.append(ff)
    for t in range(NT):
        for blk in range(NB):
            if plan[t][blk] == 'M':
                mt, fm = single([128, 1024], BF16, f"mk{t}_{blk}")
                nc.sync.dma_start(
                    mt[:], maskT[blk * 128:(blk + 1) * 128,
                                 t * 1024:(t + 1) * 1024])
                mask_tiles[(t, blk)] = mt
                mask_free.append(fm)
    # t-major so each s-half of the partial-v2 ReduceScatter fires while the
    # other half's attention units are still computing.
    h1sh = new_sh()
    v2 = new_v_all()
    q2 = [None, None]
    k2 = [None, None]
    for t in range(NT):
        for h in range(HPC):
            if t == 0 and h == 0:
                # pair-1 projections ride t0's ACT slack via the filler
                q1[1] = project_qk_pair_f(de_sb, wq_sb, q_pool, 1)
                k1[1] = project_qk_pair_f(de_sb, wk_sb, k_pool, 1)
            if t == 0 and h == 2:
                drain_all()   # pair-1 q/k must be fully written before use
            # attn2 pair-0 projections ride t1's slack so exp2 can start
            # the moment attention 1 drains (hiding RS-1)
            if t == 1 and h == 0:
                q2[0] = project_qk_pair_f(en_sb, wq_sb, q_pool, 0)
                k2[0] = project_qk_pair_f(en_sb, wk_sb, k_pool, 0)
            attn_unit(q1, k1, v1, plan, mask_tiles, h1sh, h, t)
        drain_all()
        if t == 0:
            v2_partial_half(h1sh, 0)
    for f in reversed(mask_free):
        f()
    for f in reversed(de_free):
        f()

    # ---- attention 2 (q,k from en_x; v from reduce-scattered h1@Wv) -----
    # Order on PE after the last attn1 unit: t=1 v2 partials (starts RS-1
    # early); the pair-1 projections ride attn2-h0's units via the filler.
    v2_partial_half(h1sh, 1)
    v2_fill_half(v2, 0)
    # pair-1 projections ride attn2-h0's units via the filler; the en frees
    # and the weight-tile creations that must follow them are deferred into
    # the loop at h==1 (after those filler items have drained).
    q2[1] = project_qk_pair_f(en_sb, wq_sb, q_pool, 1)
    k2[1] = project_qk_pair_f(en_sb, wk_sb, k_pool, 1)
    v2_fill_half(v2, 1)

    w1_sb, w2_sb, ff1_sb = [], [], []
    F = {}

    def setup_ffn_weights():
        # w1 is host-permuted so per-head folded A2A chunks hit contiguous
        # 128-row stationary slices
        for kc in range(KC):
            t_, _ = single([128, DFF], BF16, f"w1_{kc}")
            nc.sync.dma_start(t_[:], w1[kc * 128:(kc + 1) * 128, :])
            w1_sb.append(t_)
        F['b1'], _ = single([128, DFF // 128], F32, "b1sb")
        F['b2'], _ = single([128, DIM // 128], F32, "b2sb")
        F['zm'], _ = single([128, 2], F32, "zmsb")
        nc.sync.dma_start(F['b1'][:], b1t[:])
        nc.sync.dma_start(F['b2'][:], b2t[:])
        nc.sync.dma_start(F['zm'][:], zmask[:])
        for dc in range(DFF // 128):
            t_, _ = single([128, DIM], BF16, f"w2_{dc}")
            nc.sync.dma_start(t_[:], w2[dc * 128:(dc + 1) * 128, :])
            w2_sb.append(t_)
        for dffb in range(DFF // 128):
            t_, _ = single([128, SL], BF16, f"ff1_{dffb}")
            ff1_sb.append(t_)

    def ffn_fold(h):
        """Fold head h's A2A arrival (zmask zeroes the foreign-batch copy).
        Recv DMAs ride the Pool queue so they never head-of-line-block the
        SP queue behind a later head's A2A input DMAs."""
        fold = []
        for i in range(2):
            lo = fold_pool.tile([128, SL], BF16, tag="fl", name=f"flo{h}_{i}")
            hi = fold_pool.tile([128, SL], BF16, tag="fh", name=f"fhi{h}_{i}")
            nc.gpsimd.dma_start(lo[:], cch_out[h][i * 128:(i + 1) * 128, :])
            nc.gpsimd.dma_start(
                hi[:], cch_out[h][256 + i * 128:256 + (i + 1) * 128, :])
            nc.vector.tensor_scalar_mul(lo[:], lo[:], F['zm'][:, 0:1])
            nc.vector.tensor_scalar_mul(hi[:], hi[:], F['zm'][:, 1:2])
            nc.vector.tensor_add(lo[:], lo[:], hi[:])
            fold.append(lo)
        return fold

    def ffn_w1(h, tail):
        """Head h's W1 contribution, accumulated in-place in ff1_sb.
        tail=False emits via the filler (safe: only queued two heads after
        the A2A fired, so the data is long since landed)."""
        fold = ffn_fold(h)
        for dffb in range(DFF // 128):
            state = {}
            def it_mm(h=h, dffb=dffb, fold=fold, state=state):
                ps = ps_aux.tile([128, 512], F32, tag="aps", name="f1ps")
                state["ps"] = ps
                for i in range(2):
                    nc.tensor.matmul(
                        ps[:, 0:SL],
                        w1_sb[2 * h + i][:, dffb * 128:(dffb + 1) * 128],
                        fold[i][:],
                        start=(i == 0), stop=(i == 1))
            def it_acc(h=h, dffb=dffb, state=state):
                ps = state["ps"]
                if h == 0:
                    nc.vector.tensor_copy(ff1_sb[dffb][:], ps[:, 0:SL])
                elif h < HPC - 1:
                    nc.vector.tensor_add(ff1_sb[dffb][:], ff1_sb[dffb][:],
                                         ps[:, 0:SL])
                else:
                    nc.vector.tensor_add(ps[:, 0:SL], ps[:, 0:SL],
                                         ff1_sb[dffb][:])
                    nc.scalar.activation(ff1_sb[dffb][:], ps[:, 0:SL],
                                         AF.Relu,
                                         bias=F['b1'][:, dffb:dffb + 1])
            if tail:
                it_mm(); it_acc()
            else:
                filler.append(it_mm); filler.append(it_acc)

    # h-major so each head's A2A fires while later heads are still
    # computing; head h-2's fold+W1 pass rides the filler through head h's
    # units (two heads back => its A2A has long completed).
    h2sh = new_sh()
    for h in range(HPC):
        if h == 1:
            # pair-1 q2/k2 filler items (which read en) have drained during
            # h0's units; now en can be freed and its space reused for the
            # FFN weights.
            drain_all()
            for f in reversed(en_free):
                f()
            setup_ffn_weights()
        if h >= 2:
            ffn_w1(h - 2, tail=False)
        for t in range(NT):
            attn_unit(q2, k2, v2, noplan, {}, h2sh, h, t)
        pair, off = h // 2, (h % 2) * 64
        for j in range(N_CORES):
            nc.sync.dma_start(
                cch_in[h][j * DH:(j + 1) * DH, :],
                h2sh[pair][off:off + 64, (j % G) * SL:(j % G + 1) * SL])
        nc.gpsimd.collective_compute(
            "AllToAll", mybir.AluOpType.bypass,
            replica_groups=[[0, 1, 2, 3, 4, 5, 6, 7]],
            ins=[cch_in[h].opt()], outs=[cch_out[h].opt()])

    # ---- FFN tail on own S/4 rows --------------------------------------
    drain_all()
    ffn_w1(HPC - 2, tail=True)
    ffn_w1(HPC - 1, tail=True)
    for dimb in range(DIM // 128):
        ps = ps_aux.tile([128, 512], F32, tag="aps", name="yps")
        for dc in range(DFF // 128):
            nc.tensor.matmul(
                ps[:, 0:SL],
                w2_sb[dc][:, dimb * 128:(dimb + 1) * 128],
                ff1_sb[dc][:],
                start=(dc == 0), stop=(dc == DFF // 128 - 1))
        ysb = y_pool.tile([128, SL], F32, tag="y", name="ysb")
        nc.vector.tensor_scalar_add(ysb[:], ps[:, 0:SL],
                                    F['b2'][:, dimb:dimb + 1])
        nc.sync.dma_start(yT[dimb * 128:(dimb + 1) * 128, :], ysb[:])

    release_rest()


def _prep_inputs(de_x, en_x, mask, Wq, Wk, Wv, W1, b1, W2, b2, has_mask):
    bf = ml_dtypes.bfloat16
    scale = 1.0 / math.sqrt(DH)
    in_maps = []
    deT = [np.ascontiguousarray(de_x[b].T).astype(bf) for b in range(B)]
    enT = [np.ascontiguousarray(en_x[b].T).astype(bf) for b in range(B)]
    # W1 rows permuted so that per-head A2A fold chunks are contiguous:
    # w1p[h*256 + j*64 + r] = W1[(j*4 + h)*64 + r]
    perm = np.empty(DIM, np.int64)
    for h in range(HPC):
        for j in range(G):
            perm[h * 256 + j * 64:h * 256 + (j + 1) * 64] = \
                np.arange((j * G + h) * 64, (j * G + h) * 64 + 64)
    w1p = np.ascontiguousarray(W1[perm]).astype(bf)
    w2b = W2.astype(bf)
    b1t = np.ascontiguousarray(b1.reshape(DFF // 128, 128).T).astype(np.float32)
    b2t = np.ascontiguousarray(b2.reshape(DIM // 128, 128).T).astype(np.float32)
    wv_flat = np.ascontiguousarray(
        np.transpose(Wv, (1, 0, 2)).reshape(DIM, H * DH)).astype(bf)
    mT = None
    if has_mask:
        mT = np.ascontiguousarray(mask.T * np.float32(NEG)).astype(bf)
    for c in range(N_CORES):
        b, g = divmod(c, G)
        hs = slice(g * HPC, (g + 1) * HPC)
        m = {
            "de_xT": deT[b],
            "en_xT": enT[b],
            "wq": np.ascontiguousarray(
                np.transpose(Wq[hs] * scale, (1, 0, 2)).reshape(DIM, HPC * DH)
            ).astype(bf),
            "wk": np.ascontiguousarray(
                np.transpose(Wk[hs], (1, 0, 2)).reshape(DIM, HPC * DH)).astype(bf),
            "wv": np.ascontiguousarray(
                np.transpose(Wv[hs], (1, 0, 2)).reshape(DIM, HPC * DH)).astype(bf),
            # partial-v2 weights: rows = this core's 256 h1-features,
            # cols = all 16 heads' v outputs
            "wv2": np.ascontiguousarray(
                wv_flat[g * HPC * DH:(g + 1) * HPC * DH, :]),
            "w1": w1p, "w2": w2b, "b1t": b1t, "b2t": b2t,
        }
        zm = np.zeros((128, 2), np.float32)
        zm[:, 0 if b == 0 else 1] = 1.0
        m["zmask"] = zm
        if has_mask:
            m["maskT"] = mT
        in_maps.append(m)
    return in_maps


def get_program(mask):
    plan = _mask_plan(np.asarray(mask))
    if plan not in _CACHE:
        _CACHE[plan] = _build(plan)
    return _CACHE[plan]


_RUNNERS: dict = {}


def _fast_runner(nc):
    """Build (once) a cached jitted SPMD executor for this program.

    run_bass_kernel_spmd re-creates and re-traces its jax.jit closure on
    every call; caching the jitted shard_map shaves seconds of dispatch
    overhead off warm calls. Mirrors bass2jax.run_bass_via_pjrt.
    """
    import jax
    from jax.sharding import Mesh, PartitionSpec
    try:
        from jax.experimental.shard_map import shard_map
    except ImportError:
        from jax.shard_map import shard_map
    import concourse.mybir as _mb
    from concourse import bass2jax as b2j

    b2j.install_neuronx_cc_hook()
    partition_name = (nc.partition_id_tensor.name
                      if nc.partition_id_tensor else None)
    in_names, out_names, out_avals = [], [], []
    for alloc in nc.m.functions[0].allocations:
        if not isinstance(alloc, _mb.MemoryLocationSet):
            continue
        name = alloc.memorylocations[0].name
        if alloc.kind == "ExternalInput":
            if name != partition_name:
                in_names.append(name)
        elif alloc.kind == "ExternalOutput":
            out_names.append(name)
            out_avals.append(jax.core.ShapedArray(
                tuple(alloc.tensor_shape), _mb.dt.np(alloc.dtype)))
    n_params = len(in_names)
    n_outs = len(out_avals)
    all_names = in_names + out_names + ([partition_name] if partition_name else [])
    donate = tuple(range(n_params, n_params + n_outs))

    def _body(*args):
        operands = list(args)
        if partition_name is not None:
            operands.append(b2j.partition_id_tensor())
        return tuple(b2j._bass_exec_p.bind(
            *operands,
            out_avals=tuple(out_avals),
            in_names=tuple(all_names),
            out_names=tuple(out_names),
            lowering_input_output_aliases=(),
            sim_require_finite=True,
            sim_require_nnan=True,
            nc=nc,
        ))

    devices = jax.devices()[:N_CORES]
    mesh = Mesh(np.asarray(devices), ("core",))
    in_specs = (PartitionSpec("core"),) * (n_params + n_outs)
    out_specs = (PartitionSpec("core"),) * n_outs
    sharded = jax.jit(
        shard_map(_body, mesh=mesh, in_specs=in_specs, out_specs=out_specs,
                  check_rep=False),
        donate_argnums=donate, keep_unused=True)

    def runner(in_maps):
        concat_in = [np.concatenate([in_maps[c][nm] for c in range(N_CORES)],
                                    axis=0) for nm in in_names]
        zeros = [np.zeros((N_CORES * a.shape[0], *a.shape[1:]), a.dtype)
                 for a in out_avals]
        out_arrs = sharded(*concat_in, *zeros)
        return [
            {nm: np.asarray(out_arrs[i]).reshape(N_CORES, *out_avals[i].shape)[c]
             for i, nm in enumerate(out_names)}
            for c in range(N_CORES)
        ]

    return runner


def run(inputs, want_results=False, **run_kwargs):
    nc, has_mask = get_program(inputs["mask"])
    in_maps = _prep_inputs(
        inputs["de_x"], inputs["en_x"], inputs["mask"],
        inputs["Wq"], inputs["Wk"], inputs["Wv"],
        inputs["W1"], inputs["b1"], inputs["W2"], inputs["b2"], has_mask)
    results = None
    res = None
    if not run_kwargs:
        try:
            key = id(nc)
            if key not in _RUNNERS:
                _RUNNERS[key] = _fast_runner(nc)
            results = _RUNNERS[key](in_maps)
        except Exception:
            results = None
    if results is None:
        res = bass_utils.run_bass_kernel_spmd(
            nc, in_maps, core_ids=list(range(N_CORES)), **run_kwargs)
        results = res.results
    y = np.empty((B, S, DIM), np.float32)
    for c in range(N_CORES):
        b, g = divmod(c, G)
        y[b, g * SL:(g + 1) * SL, :] = results[c]["yT"].T
    return (y, res) if want_results else y


def kernel(**inputs) -> np.ndarray:
    return run({k: np.asarray(v) for k, v in inputs.items()})


# revision 31
# speedup vs baseline: 1.0167x; 1.0023x over previous
"""Fused decoder-layer kernel for one TRN2 chip (8 NeuronCores).

Problem (B=2, S=2048, DIM=1024, H=16, DH=64, DFF=2048):
    h1 = MHA(q=de_x, k=de_x, v=de_x, mask)   (shared per-head weights Wq/Wk/Wv)
    h2 = MHA(q=en_x, k=en_x, v=h1,  None)
    y  = relu(h2 @ W1 + b1) @ W2 + b2

Sharding: core c = 4*b + g  (b = batch, g = head-group of 4 heads; g also
indexes the S/4 slice of rows this core runs the FFN on).

Layout strategy (all matmul operands pre-transposed so every contraction has
its reduction dim on SBUF partitions; all matmul inputs bf16, PSUM f32):
  - host passes x^T [DIM, S] per batch; projections produce q^T/k^T [DH, S]
    and v [S, DH] directly.
  - scores are computed transposed ([s2, s1]) so P^T feeds the PV matmul as
    stationary weights with K = s2 on partitions.
  - a ones-column appended to v makes the PV matmul also produce softmax row
    sums (flash-attention style); normalization happens on the [DH, S] output.
  - FFN runs column-transposed (ff1^T, y^T) so b1/b2 are per-partition ACT
    biases and no transposes are ever needed. Host un-transposes y^T.
Cross-core (v2 — collective-latency-optimized):
  - v2 = h1 @ Wv is computed as per-core PARTIAL products over the 256
    h1-features this core owns, then a 4-way ReduceScatter (add) within each
    batch group sums them and hands each core its own heads' 256 v-columns.
    Split into two s-halves so the first RS hides under attention-1 t=1.
  - h2^T is exchanged with four per-head 8-way AllToAlls, each fired as soon
    as that head's units finish so the wire time hides under the remaining
    attention compute. Foreign-batch rows are zeroed/folded via a tiny
    per-core 2-column mask; W1 is host-permuted so each folded head chunk
    contracts with a contiguous 128-row stationary slice, letting the FFN
    first layer accumulate incrementally per head.
  - PSUM: scores 2x[128,1024] + PV accumulator 1x[65,1024] + aux 2x[128,512]
    (projections / v2-partials / FFN) = exactly 8 banks. PSUM->SBUF copies
    run on the idle Pool (gpsimd) engine to keep DVE free for normalization.
"""

import math

import numpy as np
import ml_dtypes

import concourse.bass as bass
import concourse.bacc as bacc
import concourse.mybir as mybir
import concourse.tile as tile
import concourse.bass_utils as bass_utils

B, S, DIM, H = 2, 2048, 1024, 16
DH = DIM // H            # 64
DFF = 2048
NEG = -1.0e9
N_CORES = 8
G = 4                    # cores per batch group == head-groups == s-groups
HPC = H // G             # heads per core = 4
SL = S // G              # FFN rows per core = 512
KC = DIM // 128          # 8 k-chunks of the model dim
NB = S // 128            # 16 key blocks
NT = S // 1024           # 2 query tiles of 1024
BF16 = mybir.dt.bfloat16
F32 = mybir.dt.float32
AF = mybir.ActivationFunctionType

_CACHE: dict = {}


def _mask_plan(mask: np.ndarray):
    """Classify each [1024 x 128] (s1, s2) block: 'N' no-op, 'M' apply."""
    plan = []
    for t in range(NT):
        row = []
        for blk in range(NB):
            sub = mask[t * 1024:(t + 1) * 1024, blk * 128:(blk + 1) * 128]
            # NOTE: an 'S' (skip-block) fast path deadlocked on hardware;
            # fully-masked blocks share one all-NEG tile ('F', exp -> 0).
            row.append('N' if not sub.any() else ('F' if sub.all() else 'M'))
        plan.append(tuple(row))
    return tuple(plan)


def _build(plan):
    has_mask = any(c in 'MF' for row in plan for c in row)
    nc = bacc.Bacc("TRN2", target_bir_lowering=False, debug=False,
                   num_devices=N_CORES)

    de_xT = nc.dram_tensor("de_xT", [DIM, S], BF16, kind="ExternalInput")
    en_xT = nc.dram_tensor("en_xT", [DIM, S], BF16, kind="ExternalInput")
    wq = nc.dram_tensor("wq", [DIM, HPC * DH], BF16, kind="ExternalInput")
    wk = nc.dram_tensor("wk", [DIM, HPC * DH], BF16, kind="ExternalInput")
    wv = nc.dram_tensor("wv", [DIM, HPC * DH], BF16, kind="ExternalInput")
    wv2 = nc.dram_tensor("wv2", [HPC * DH, DIM], BF16, kind="ExternalInput")
    w1 = nc.dram_tensor("w1", [DIM, DFF], BF16, kind="ExternalInput")
    w2 = nc.dram_tensor("w2", [DFF, DIM], BF16, kind="ExternalInput")
    b1t = nc.dram_tensor("b1t", [128, DFF // 128], F32, kind="ExternalInput")
    b2t = nc.dram_tensor("b2t", [128, DIM // 128], F32, kind="ExternalInput")
    zmask = nc.dram_tensor("zmask", [128, 2], F32, kind="ExternalInput")
    maskT = None
    if has_mask:
        maskT = nc.dram_tensor("maskT", [S, S], BF16, kind="ExternalInput")
    yT = nc.dram_tensor("yT", [DIM, SL], F32, kind="ExternalOutput")

    with tile.TileContext(nc) as tc:
        _trace(nc, tc, plan, de_xT, en_xT, wq, wk, wv, wv2, w1, w2, b1t, b2t,
               zmask, maskT, yT)
    nc.compile()
    return nc, has_mask


def _trace(nc, tc, plan, de_xT, en_xT, wq, wk, wv, wv2, w1, w2, b1t, b2t,
           zmask, maskT, yT):
    # Unified allocation stack: everything (pools and single tiles) must be
    # released in strict LIFO order before TileContext exits.
    stack = nc._tile_stack = []   # [release_fn or None(done)]

    def _push(release_fn):
        ent = {"f": release_fn}
        stack.append(ent)
        def rel():
            assert ent["f"] is not None, "double release"
            ent["f"](); ent["f"] = None
        return rel

    def release_rest():
        for ent in reversed(stack):
            if ent["f"] is not None:
                ent["f"](); ent["f"] = None

    noplan = tuple(tuple('N' for _ in range(NB)) for _ in range(NT))

    # ---- pools ----------------------------------------------------------
    def pool(**kw):
        cm = tc.tile_pool(**kw)
        p = cm.__enter__()
        _push(lambda: cm.__exit__(None, None, None))
        return p

    def single(shape, dtype, name):
        t_, f_ = tc.tile(shape, dtype, name=name)
        return t_, _push(f_)

    has_mask = any(c in 'MF' for row in plan for c in row)
    ps_sc = pool(name="ps_sc", bufs=2, space="PSUM")     # [128,1024] scores/proj
    ps_hx = pool(name="ps_hx", bufs=1, space="PSUM")     # [65,1024] PV accum
    ps_aux = pool(name="ps_aux", bufs=2, space="PSUM")   # [128,512] v2p/FFN
    # pt depth = exp run-ahead across the RS-1 latency gap; the masked build
    # spends 64KB/partition on mask tiles so it gets a shallower pool.
    pt_pool = pool(name="pt", bufs=6 if has_mask else 17)
    rc_pool = pool(name="rc", bufs=1)
    bc_pool = pool(name="bc", bufs=1)
    # bufs=3: q1 pair-0/1 plus the filler-projected q2 pair-0 coexist
    q_pool = pool(name="qp", bufs=3)
    k_pool = pool(name="kp", bufs=3)
    v_pool = pool(name="vp", bufs=2)
    sh_pool = pool(name="shp", bufs=2)
    y_pool = pool(name="yp", bufs=1)
    fold_pool = pool(name="fold", bufs=4)
    dram = pool(name="dram", bufs=1, space="DRAM")

    # ---- persistent tiles (stack order: frees must pop LIFO) ------------
    wq_sb, _ = single([128, KC, HPC * DH], BF16, "wqsb")
    wk_sb, _ = single([128, KC, HPC * DH], BF16, "wksb")
    wv_sb, _ = single([128, KC, HPC * DH], BF16, "wvsb")
    wv2_sb, _ = single([128, 2, DIM], BF16, "wv2sb")
    # warm the ACT exp table while input DMAs stream (allocated below the
    # en/de stack so it is never popped before them)
    warm_sb, _ = single([128, 8], F32, "warm")
    nc.vector.memset(warm_sb[:], 0.0)
    nc.scalar.activation(warm_sb[:], warm_sb[:], AF.Exp)
    # en allocated BELOW de on the stack: de is freed first (after attn1),
    # en later (after attn2 q/k projections).
    en_sb, en_free = [], []
    for kc in range(KC):
        t_, f_ = single([128, S], BF16, f"en{kc}")
        en_sb.append(t_); en_free.append(f_)
    de_sb, de_free = [], []
    for kc in range(KC):
        t_, f_ = single([128, S], BF16, f"de{kc}")
        de_sb.append(t_); de_free.append(f_)
    # DMA trace order: attn1's operands first
    nc.sync.dma_start(wq_sb[:], wq.rearrange("(a p) c -> p a c", p=128))
    nc.sync.dma_start(wk_sb[:], wk.rearrange("(a p) c -> p a c", p=128))
    for kc in range(KC):
        nc.sync.dma_start(de_sb[kc][:], de_xT[kc * 128:(kc + 1) * 128, :])
    nc.sync.dma_start(wv_sb[:], wv.rearrange("(a p) c -> p a c", p=128))
    nc.sync.dma_start(wv2_sb[:], wv2.rearrange("(a p) c -> p a c", p=128))
    for kc in range(KC):
        nc.sync.dma_start(en_sb[kc][:], en_xT[kc * 128:(kc + 1) * 128, :])

    # collective bounce buffers.
    # v2 ReduceScatter, one per s-half: in = 4 chunks (head-groups) of
    # [1024 s, 256 e]; out = this core's summed [1024 s, 256 e].
    ccv_in = [dram.tile([G * 1024, HPC * DH], BF16, name=f"ccvi{t}")
              for t in range(NT)]
    ccv_out = [dram.tile([1024, HPC * DH], BF16, name=f"ccvo{t}")
               for t in range(NT)]
    # per-head h2 AllToAll: in = 8 chunks of [64 f, 512 s]; out = 8 blocks.
    cch_in = [dram.tile([N_CORES * DH, SL], BF16, name=f"cchi{h}")
              for h in range(HPC)]
    cch_out = [dram.tile([N_CORES * DH, SL], BF16, name=f"ccho{h}")
               for h in range(HPC)]

    # ---- helpers --------------------------------------------------------
    # Filler queue: small PE work items drained one per score-block inside
    # attention units. ACT paces attention (~1.1us/block) while PE only needs
    # ~0.85us, so ~1 extra matmul per block rides for free instead of a
    # projection/FFN block stalling the exp pipeline for 7-15us.
    filler = []

    def drain_filler(k):
        for _ in range(k):
            if not filler:
                return
            filler.pop(0)()

    def drain_all():
        while filler:
            filler.pop(0)()

    def project_qk_pair(x_sb, w_sb, pool, pair):
        """q^T (or k^T) for one head pair as a [128, S] bf16 tile."""
        qt = pool.tile([128, S], BF16, tag="qk", name=f"qk{pair}")
        for st in range(NT):
            ps = ps_sc.tile([128, 1024], F32, tag="ps", name="pjps")
            for kc in range(KC):
                for nn in (0, 512):
                    nc.tensor.matmul(
                        ps[:, nn:nn + 512],
                        w_sb[:, kc, pair * 128:(pair + 1) * 128],
                        x_sb[kc][:, st * 1024 + nn:st * 1024 + nn + 512],
                        start=(kc == 0), stop=(kc == KC - 1))
            nc.vector.tensor_copy(qt[:, st * 1024:(st + 1) * 1024], ps[:])
        return qt

    def project_qk_pair_f(x_sb, w_sb, pool, pair):
        """Filler variant: emits the projection as 512-col chunks of 2
        matmuls per item into the aux PSUM pool; returns the tile handle
        immediately (writes land as the filler drains)."""
        qt = pool.tile([128, S], BF16, tag="qk", name=f"qkf{pair}")
        for st in range(NT):
            for nn in (0, 512):
                state = {}
                for kc0 in range(0, KC, 2):
                    def item(st=st, nn=nn, kc0=kc0, state=state):
                        if kc0 == 0:
                            state["ps"] = ps_aux.tile(
                                [128, 512], F32, tag="aps", name="pjf")
                        for kc in (kc0, kc0 + 1):
                            nc.tensor.matmul(
                                state["ps"][:],
                                w_sb[:, kc, pair * 128:(pair + 1) * 128],
                                x_sb[kc][:, st * 1024 + nn:
                                          st * 1024 + nn + 512],
                                start=(kc == 0), stop=(kc == KC - 1))
                    filler.append(item)
                def fin(st=st, nn=nn, state=state):
                    nc.vector.tensor_copy(
                        qt[:, st * 1024 + nn:st * 1024 + nn + 512],
                        state["ps"][:])
                filler.append(fin)
        return qt

    def project_v(src_sb, v_all, blk_lo, blk_hi):
        """v for 4 heads + ones column into v_all[128, NB*HPC*65] (bf16)."""
        for blk in range(blk_lo, blk_hi):
            ps = ps_sc.tile([128, 1024], F32, tag="ps", name="vps")
            for kc in range(KC):
                nc.tensor.matmul(
                    ps[:, 0:HPC * DH],
                    src_sb[kc][:, blk * 128:(blk + 1) * 128],
                    wv_sb[:, kc, :],
                    start=(kc == 0), stop=(kc == KC - 1))
            for h in range(HPC):
                nc.vector.tensor_copy(
                    v_all[:, (blk * HPC + h) * 65:(blk * HPC + h) * 65 + 64],
                    ps[:, h * DH:(h + 1) * DH])

    def new_v_all():
        v_all = v_pool.tile([128, NB * HPC * 65], BF16, tag="v", name="vall")
        v3 = v_all[:].rearrange("p (b c) -> p b c", c=65)
        nc.vector.memset(v3[:, :, 64:65], 1.0)
        return v_all

    def attn_unit(q_pairs, k_pairs, v_all, aplan, mask_tiles_in, sh, h, t):
        """scores -> exp -> PV -> normalize for one (head, query-tile)."""
        pair, off = h // 2, (h % 2) * 64
        hext = ps_hx.tile([65, 1024], F32, name="hext")
        for blk in range(NB):
            sc = ps_sc.tile([128, 1024], F32, tag="ps", name="scps")
            for nn in (0, 512):
                nc.tensor.matmul(
                    sc[:, nn:nn + 512],
                    k_pairs[pair][off:off + 64, blk * 128:(blk + 1) * 128],
                    q_pairs[pair][off:off + 64,
                                  t * 1024 + nn:t * 1024 + nn + 512],
                    start=True, stop=True)
            if aplan[t][blk] == 'M':
                nc.vector.tensor_add(sc[:], sc[:], mask_tiles_in[(t, blk)][:])
            elif aplan[t][blk] == 'F':
                nc.vector.tensor_add(sc[:], sc[:], mask_tiles_in['F'][:])
            pt = pt_pool.tile([128, 1024], BF16, name="pt")
            nc.scalar.activation(pt[:], sc[:], AF.Exp)
            vs = v_all[:, (blk * HPC + h) * 65:(blk * HPC + h) * 65 + 65]
            for nn in (0, 512):
                nc.tensor.matmul(
                    hext[:, nn:nn + 512], vs, pt[:, nn:nn + 512],
                    start=(blk == 0), stop=(blk == NB - 1))
            drain_filler(2 if len(filler) > 28 else 1)
        recip = rc_pool.tile([1, 1024], F32, name="recip")
        nc.vector.reciprocal(recip[:], hext[64:65, :])
        rbc = bc_pool.tile([64, 1024], F32, name="rbc")
        nc.gpsimd.partition_broadcast(rbc[:], recip[0:1, :])
        nc.vector.tensor_mul(
            sh[pair][off:off + 64, t * 1024:(t + 1) * 1024],
            hext[0:64, :], rbc[:])

    def new_sh():
        return [sh_pool.tile([128, S], BF16, tag="sh", name=f"sh{p}")
                for p in range(2)]

    def v2_partial_half(h1sh, t):
        """Partial v2 (own 256 h1 features x all 1024 v-cols) for s-half t,
        DMA'd into the RS bounce buffer as 4 head-group chunks."""
        for sb in range(8):
            col = t * 1024 + sb * 128
            vp = fold_pool.tile([128, DIM], BF16, tag="v2p", name="v2p")
            for eh in (0, 512):
                ps = ps_aux.tile([128, 512], F32, tag="aps", name="v2ps")
                for pair in range(2):
                    nc.tensor.matmul(
                        ps[:],
                        h1sh[pair][:, col:col + 128],
                        wv2_sb[:, pair, eh:eh + 512],
                        start=(pair == 0), stop=(pair == 1))
                # DVE, not Pool: the q2/k2 projection copies ride the Pool
                # queue and must not stall behind these
                nc.vector.tensor_copy(vp[:, eh:eh + 512], ps[:])
            for g in range(G):
                nc.sync.dma_start(
                    ccv_in[t][g * 1024 + sb * 128:g * 1024 + (sb + 1) * 128, :],
                    vp[:, g * HPC * DH:(g + 1) * HPC * DH])
        nc.gpsimd.collective_compute(
            "ReduceScatter", mybir.AluOpType.add,
            replica_groups=[[0, 1, 2, 3], [4, 5, 6, 7]],
            ins=[ccv_in[t].opt()], outs=[ccv_out[t].opt()])

    def v2_fill_half(v_all, t):
        """DMA the reduce-scattered v2 s-half into v_all's per-head slots."""
        v3 = v_all[:].rearrange("p (b h c) -> p b h c", h=HPC, c=65)
        for sb in range(8):
            blk = t * 8 + sb
            nc.sync.dma_start(
                v3[:, blk, :, 0:64],
                ccv_out[t][sb * 128:(sb + 1) * 128, :]
                .rearrange("p (h c) -> p h c", c=64))

    # ---- attention 1 (self-attn on de_x, mask) --------------------------
    # pair-0 q/k and v project first so the exp pipeline (ACT) starts as
    # early as possible; pair-1 projections slot in behind the first units.
    q1 = [None, None]
    k1 = [None, None]
    q1[0] = project_qk_pair(de_sb, wq_sb, q_pool, 0)
    k1[0] = project_qk_pair(de_sb, wk_sb, k_pool, 0)
    v1 = new_v_all()
    project_v(de_sb, v1, 0, NB)
    mask_tiles, mask_free = {}, []
    if any(c == 'F' for row in plan for c in row):
        ft, ff = single([128, 1024], BF16, "mkF")
        nc.vector.memset(ft[:], NEG)
        mask_tiles['F'] = ft
        mask_free.append(ff)
    for t in range(NT):
        for blk in range(NB):
            if plan[t][blk] == 'M':
                mt, fm = single([128, 1024], BF16, f"mk{t}_{blk}")
                nc.sync.dma_start(
                    mt[:], maskT[blk * 128:(blk + 1) * 128,
                                 t * 1024:(t + 1) * 1024])
                mask_tiles[(t, blk)] = mt
                mask_free.append(fm)
    # t-major so each s-half of the partial-v2 ReduceScatter fires while the
    # other half's attention units are still computing.
    h1sh = new_sh()
    v2 = new_v_all()
    q2 = [None, None]
    k2 = [None, None]
    for t in range(NT):
        for h in range(HPC):
            if t == 0 and h == 0:
                # pair-1 projections ride t0's ACT slack via the filler
                q1[1] = project_qk_pair_f(de_sb, wq_sb, q_pool, 1)
                k1[1] = project_qk_pair_f(de_sb, wk_sb, k_pool, 1)
            if t == 0 and h == 2:
                drain_all()   # pair-1 q/k must be fully written before use
            # attn2 pair-0 projections ride t1's slack so exp2 can start
            # the moment attention 1 drains (hiding RS-1)
            if t == 1 and h == 0:
                q2[0] = project_qk_pair_f(en_sb, wq_sb, q_pool, 0)
                k2[0] = project_qk_pair_f(en_sb, wk_sb, k_pool, 0)
            attn_unit(q1, k1, v1, plan, mask_tiles, h1sh, h, t)
        drain_all()
        if t == 0:
            v2_partial_half(h1sh, 0)
    for f in reversed(mask_free):
        f()
    for f in reversed(de_free):
        f()

    # ---- attention 2 (q,k from en_x; v from reduce-scattered h1@Wv) -----
    # Order on PE after the last attn1 unit: t=1 v2 partials (starts RS-1
    # early); the pair-1 projections ride attn2-h0's units via the filler.
    v2_partial_half(h1sh, 1)
    v2_fill_half(v2, 0)
    # pair-1 projections ride attn2-h0's units via the filler; the en frees
    # and the weight-tile creations that must follow them are deferred into
    # the loop at h==1 (after those filler items have drained).
    q2[1] = project_qk_pair_f(en_sb, wq_sb, q_pool, 1)
    k2[1] = project_qk_pair_f(en_sb, wk_sb, k_pool, 1)
    v2_fill_half(v2, 1)

    w1_sb, w2_sb, ff1_sb = [], [], []
    F = {}

    def setup_ffn_weights():
        # w1 is host-permuted so per-head folded A2A chunks hit contiguous
        # 128-row stationary slices
        for kc in range(KC):
            t_, _ = single([128, DFF], BF16, f"w1_{kc}")
            nc.sync.dma_start(t_[:], w1[kc * 128:(kc + 1) * 128, :])
            w1_sb.append(t_)
        F['b1'], _ = single([128, DFF // 128], F32, "b1sb")
        F['b2'], _ = single([128, DIM // 128], F32, "b2sb")
        F['zm'], _ = single([128, 2], F32, "zmsb")
        nc.sync.dma_start(F['b1'][:], b1t[:])
        nc.sync.dma_start(F['b2'][:], b2t[:])
        nc.sync.dma_start(F['zm'][:], zmask[:])
        for dc in range(DFF // 128):
            t_, _ = single([128, DIM], BF16, f"w2_{dc}")
            nc.sync.dma_start(t_[:], w2[dc * 128:(dc + 1) * 128, :])
            w2_sb.append(t_)
        for dffb in range(DFF // 128):
            t_, _ = single([128, SL], BF16, f"ff1_{dffb}")
            ff1_sb.append(t_)

    def ffn_fold(h):
        """Fold head h's A2A arrival (zmask zeroes the foreign-batch copy).
        Recv DMAs ride the Pool queue so they never head-of-line-block the
        SP queue behind a later head's A2A input DMAs."""
        fold = []
        for i in range(2):
            lo = fold_pool.tile([128, SL], BF16, tag="fl", name=f"flo{h}_{i}")
            hi = fold_pool.tile([128, SL], BF16, tag="fh", name=f"fhi{h}_{i}")
            nc.gpsimd.dma_start(lo[:], cch_out[h][i * 128:(i + 1) * 128, :])
            nc.gpsimd.dma_start(
                hi[:], cch_out[h][256 + i * 128:256 + (i + 1) * 128, :])
            nc.vector.tensor_scalar_mul(lo[:], lo[:], F['zm'][:, 0:1])
            nc.vector.tensor_scalar_mul(hi[:], hi[:], F['zm'][:, 1:2])
            nc.vector.tensor_add(lo[:], lo[:], hi[:])
            fold.append(lo)
        return fold

    def ffn_w1(h, tail):
        """Head h's W1 contribution, accumulated in-place in ff1_sb.
        tail=False emits via the filler (safe: only queued two heads after
        the A2A fired, so the data is long since landed)."""
        fold = ffn_fold(h)
        for dffb in range(DFF // 128):
            state = {}
            def it_mm(h=h, dffb=dffb, fold=fold, state=state):
                ps = ps_aux.tile([128, 512], F32, tag="aps", name="f1ps")
                state["ps"] = ps
                for i in range(2):
                    nc.tensor.matmul(
                        ps[:, 0:SL],
                        w1_sb[2 * h + i][:, dffb * 128:(dffb + 1) * 128],
                        fold[i][:],
                        start=(i == 0), stop=(i == 1))
            def it_acc(h=h, dffb=dffb, state=state):
                ps = state["ps"]
                if h == 0:
                    nc.vector.tensor_copy(ff1_sb[dffb][:], ps[:, 0:SL])
                elif h < HPC - 1:
                    nc.vector.tensor_add(ff1_sb[dffb][:], ff1_sb[dffb][:],
                                         ps[:, 0:SL])
                else:
                    nc.vector.tensor_add(ps[:, 0:SL], ps[:, 0:SL],
                                         ff1_sb[dffb][:])
                    nc.scalar.activation(ff1_sb[dffb][:], ps[:, 0:SL],
                                         AF.Relu,
                                         bias=F['b1'][:, dffb:dffb + 1])
            if tail:
                it_mm(); it_acc()
            else:
                filler.append(it_mm); filler.append(it_acc)

    # h-major so each head's A2A fires while later heads are still
    # computing; head h-2's fold+W1 pass rides the filler through head h's
    # units (two heads back => its A2A has long completed).
    h2sh = new_sh()
    for h in range(HPC):
        if h == 1:
            # pair-1 q2/k2 filler items (which read en) have drained during
            # h0's units; now en can be freed and its space reused for the
            # FFN weights.
            drain_all()
            for f in reversed(en_free):
                f()
            setup_ffn_weights()
        if h >= 2:
            ffn_w1(h - 2, tail=False)
        for t in range(NT):
            attn_unit(q2, k2, v2, noplan, {}, h2sh, h, t)
        pair, off = h // 2, (h % 2) * 64
        for j in range(N_CORES):
            nc.sync.dma_start(
                cch_in[h][j * DH:(j + 1) * DH, :],
                h2sh[pair][off:off + 64, (j % G) * SL:(j % G + 1) * SL])
        nc.gpsimd.collective_compute(
            "AllToAll", mybir.AluOpType.bypass,
            replica_groups=[[0, 1, 2, 3, 4, 5, 6, 7]],
            ins=[cch_in[h].opt()], outs=[cch_out[h].opt()])

    # ---- FFN tail on own S/4 rows --------------------------------------
    drain_all()
    ffn_w1(HPC - 2, tail=True)
    ffn_w1(HPC - 1, tail=True)
    for dimb in range(DIM // 128):
        ps = ps_aux.tile([128, 512], F32, tag="aps", name="yps")
        for dc in range(DFF // 128):
            nc.tensor.matmul(
                ps[:, 0:SL],
                w2_sb[dc][:, dimb * 128:(dimb + 1) * 128],
                ff1_sb[dc][:],
                start=(dc == 0), stop=(dc == DFF // 128 - 1))
        ysb = y_pool.tile([128, SL], F32, tag="y", name="ysb")
        nc.vector.tensor_scalar_add(ysb[:], ps[:, 0:SL],
                                    F['b2'][:, dimb:dimb + 1])
        nc.sync.dma_start(yT[dimb * 128:(dimb + 1) * 128, :], ysb[:])

    release_rest()


def _prep_inputs(de_x, en_x, mask, Wq, Wk, Wv, W1, b1, W2, b2, has_mask):
    bf = ml_dtypes.bfloat16
    scale = 1.0 / math.sqrt(DH)
    in_maps = []
    deT = [np.ascontiguousarray(de_x[b].T).astype(bf) for b in range(B)]
    enT = [np.ascontiguousarray(en_x[b].T).astype(bf) for b in range(B)]
    # W1 rows permuted so that per-head A2A fold chunks are contiguous:
    # w1p[h*256 + j*64 + r] = W1[(j*4 + h)*64 + r]
    perm = np.empty(DIM, np.int64)
    for h in range(HPC):
        for j in range(G):
            perm[h * 256 + j * 64:h * 256 + (j + 1) * 64] = \
                np.arange((j * G + h) * 64, (j * G + h) * 64 + 64)
    w1p = np.ascontiguousarray(W1[perm]).astype(bf)
    w2b = W2.astype(bf)
    b1t = np.ascontiguousarray(b1.reshape(DFF // 128, 128).T).astype(np.float32)
    b2t = np.ascontiguousarray(b2.reshape(DIM // 128, 128).T).astype(np.float32)
    wv_flat = np.ascontiguousarray(
        np.transpose(Wv, (1, 0, 2)).reshape(DIM, H * DH)).astype(bf)
    mT = None
    if has_mask:
        mT = np.ascontiguousarray(mask.T * np.float32(NEG)).astype(bf)
    for c in range(N_CORES):
        b, g = divmod(c, G)
        hs = slice(g * HPC, (g + 1) * HPC)
        m = {
            "de_xT": deT[b],
            "en_xT": enT[b],
            "wq": np.ascontiguousarray(
                np.transpose(Wq[hs] * scale, (1, 0, 2)).reshape(DIM, HPC * DH)
            ).astype(bf),
            "wk": np.ascontiguousarray(
                np.transpose(Wk[hs], (1, 0, 2)).reshape(DIM, HPC * DH)).astype(bf),
            "wv": np.ascontiguousarray(
                np.transpose(Wv[hs], (1, 0, 2)).reshape(DIM, HPC * DH)).astype(bf),
            # partial-v2 weights: rows = this core's 256 h1-features,
            # cols = all 16 heads' v outputs
            "wv2": np.ascontiguousarray(
                wv_flat[g * HPC * DH:(g + 1) * HPC * DH, :]),
            "w1": w1p, "w2": w2b, "b1t": b1t, "b2t": b2t,
        }
        zm = np.zeros((128, 2), np.float32)
        zm[:, 0 if b == 0 else 1] = 1.0
        m["zmask"] = zm
        if has_mask:
            m["maskT"] = mT
        in_maps.append(m)
    return in_maps


def get_program(mask):
    plan = _mask_plan(np.asarray(mask))
    if plan not in _CACHE:
        _CACHE[plan] = _build(plan)
    return _CACHE[plan]


_RUNNERS: dict = {}


def _fast_runner(nc):
    """Build (once) a cached jitted SPMD executor for this program.

    run_bass_kernel_spmd re-creates and re-traces its jax.jit closure on
    every call; caching the jitted shard_map shaves seconds of dispatch
    overhead off warm calls. Mirrors bass2jax.run_bass_via_pjrt.
    """
    import jax
    from jax.sharding import Mesh, PartitionSpec
    try:
        from jax.experimental.shard_map import shard_map
    except ImportError:
        from jax.shard_map import shard_map
    import concourse.mybir as _mb
    from concourse import bass2jax as b2j

    b2j.install_neuronx_cc_hook()
    partition_name = (nc.partition_id_tensor.name
                      if nc.partition_id_tensor else None)
    in_names, out_names, out_avals = [], [], []
    for alloc in nc.m.functions[0].allocations:
        if not isinstance(alloc, _mb.MemoryLocationSet):
            continue
        name = alloc.memorylocations[0].name
        if alloc.kind == "ExternalInput":
            if name != partition_name:
                in_names.append(name)
        elif alloc.kind == "ExternalOutput":
            out_names.append(name)
            out_avals.append(jax.core.ShapedArray(
                tuple(alloc.tensor_shape), _mb.dt.np(alloc.dtype)))
    n_params = len(in_names)
    n_outs = len(out_avals)
    all_names = in_names + out_names + ([partition_name] if partition_name else [])
    donate = tuple(range(n_params, n_params + n_outs))

    def _body(*args):
        operands = list(args)
        if partition_name is not None:
            operands.append(b2j.partition_id_tensor())
        return tuple(b2j._bass_exec_p.bind(
            *operands,
            out_avals=tuple(out_avals),
            in_names=tuple(all_names),
            out_names=tuple(out_names),
            lowering_input_output_aliases=(),
            sim_require_finite=True,
            sim_require_nnan=True,
            nc=nc,
        ))

    devices = jax.devices()[:N_CORES]
    mesh = Mesh(np.asarray(devices), ("core",))
    in_specs = (PartitionSpec("core"),) * (n_params + n_outs)
    out_specs = (PartitionSpec("core"),) * n_outs
    sharded = jax.jit(
        shard_map(_body, mesh=mesh, in_specs=in_specs, out_specs=out_specs,
                  check_rep=False),
        donate_argnums=donate, keep_unused=True)

    def runner(in_maps):
        concat_in = [np.concatenate([in_maps[c][nm] for c in range(N_CORES)],
                                    axis=0) for nm in in_names]
        zeros = [np.zeros((N_CORES * a.shape[0], *a.shape[1:]), a.dtype)
                 for a in out_avals]
        out_arrs = sharded(*concat_in, *zeros)
        return [
            {nm: np.asarray(out_arrs[i]).reshape(N_CORES, *out_avals[i].shape)[c]
             for i, nm in enumerate(out_names)}
            for c in range(N_CORES)
        ]

    return runner


def run(inputs, want_results=False, **run_kwargs):
    nc, has_mask = get_program(inputs["mask"])
    in_maps = _prep_inputs(
        inputs["de_x"], inputs["en_x"], inputs["mask"],
        inputs["Wq"], inputs["Wk"], inputs["Wv"],
        inputs["W1"], inputs["b1"], inputs["W2"], inputs["b2"], has_mask)
    results = None
    res = None
    if not run_kwargs:
        try:
            key = id(nc)
            if key not in _RUNNERS:
                _RUNNERS[key] = _fast_runner(nc)
            results = _RUNNERS[key](in_maps)
        except Exception:
            results = None
    if results is None:
        res = bass_utils.run_bass_kernel_spmd(
            nc, in_maps, core_ids=list(range(N_CORES)), **run_kwargs)
        results = res.results
    y = np.empty((B, S, DIM), np.float32)
    for c in range(N_CORES):
        b, g = divmod(c, G)
        y[b, g * SL:(g + 1) * SL, :] = results[c]["yT"].T
    return (y, res) if want_results else y


def kernel(**inputs) -> np.ndarray:
    return run({k: np.asarray(v) for k, v in inputs.items()})


# revision 34
# speedup vs baseline: 1.0197x; 1.0029x over previous
"""Fused decoder-layer kernel for one TRN2 chip (8 NeuronCores).

Problem (B=2, S=2048, DIM=1024, H=16, DH=64, DFF=2048):
    h1 = MHA(q=de_x, k=de_x, v=de_x, mask)   (shared per-head weights Wq/Wk/Wv)
    h2 = MHA(q=en_x, k=en_x, v=h1,  None)
    y  = relu(h2 @ W1 + b1) @ W2 + b2

Sharding: core c = 4*b + g  (b = batch, g = head-group of 4 heads; g also
indexes the S/4 slice of rows this core runs the FFN on).

Layout strategy (all matmul operands pre-transposed so every contraction has
its reduction dim on SBUF partitions; all matmul inputs bf16, PSUM f32):
  - host passes x^T [DIM, S] per batch; projections produce q^T/k^T [DH, S]
    and v [S, DH] directly.
  - scores are computed transposed ([s2, s1]) so P^T feeds the PV matmul as
    stationary weights with K = s2 on partitions.
  - a ones-column appended to v makes the PV matmul also produce softmax row
    sums (flash-attention style); normalization happens on the [DH, S] output.
  - FFN runs column-transposed (ff1^T, y^T) so b1/b2 are per-partition ACT
    biases and no transposes are ever needed. Host un-transposes y^T.
Cross-core (v2 — collective-latency-optimized):
  - v2 = h1 @ Wv is computed as per-core PARTIAL products over the 256
    h1-features this core owns, then a 4-way ReduceScatter (add) within each
    batch group sums them and hands each core its own heads' 256 v-columns.
    Split into two s-halves so the first RS hides under attention-1 t=1.
  - h2^T is exchanged with four per-head 8-way AllToAlls, each fired as soon
    as that head's units finish so the wire time hides under the remaining
    attention compute. Foreign-batch rows are zeroed/folded via a tiny
    per-core 2-column mask; W1 is host-permuted so each folded head chunk
    contracts with a contiguous 128-row stationary slice, letting the FFN
    first layer accumulate incrementally per head.
  - PSUM: scores 2x[128,1024] + PV accumulator 1x[65,1024] + aux 2x[128,512]
    (projections / v2-partials / FFN) = exactly 8 banks. PSUM->SBUF copies
    run on the idle Pool (gpsimd) engine to keep DVE free for normalization.
"""

import math

import numpy as np
import ml_dtypes

import concourse.bass as bass
import concourse.bacc as bacc
import concourse.mybir as mybir
import concourse.tile as tile
import concourse.bass_utils as bass_utils

B, S, DIM, H = 2, 2048, 1024, 16
DH = DIM // H            # 64
DFF = 2048
NEG = -1.0e9
N_CORES = 8
G = 4                    # cores per batch group == head-groups == s-groups
HPC = H // G             # heads per core = 4
SL = S // G              # FFN rows per core = 512
KC = DIM // 128          # 8 k-chunks of the model dim
NB = S // 128            # 16 key blocks
NT = S // 1024           # 2 query tiles of 1024
BF16 = mybir.dt.bfloat16
F32 = mybir.dt.float32
AF = mybir.ActivationFunctionType

_CACHE: dict = {}


def _mask_plan(mask: np.ndarray):
    """Classify each [1024 x 128] (s1, s2) block: 'N' no-op, 'M' apply."""
    plan = []
    for t in range(NT):
        row = []
        for blk in range(NB):
            sub = mask[t * 1024:(t + 1) * 1024, blk * 128:(blk + 1) * 128]
            # NOTE: an 'S' (skip-block) fast path deadlocked on hardware;
            # fully-masked blocks share one all-NEG tile ('F', exp -> 0).
            row.append('N' if not sub.any() else ('F' if sub.all() else 'M'))
        plan.append(tuple(row))
    return tuple(plan)


def _build(plan):
    has_mask = any(c in 'MF' for row in plan for c in row)
    nc = bacc.Bacc("TRN2", target_bir_lowering=False, debug=False,
                   num_devices=N_CORES)

    de_xT = nc.dram_tensor("de_xT", [DIM, S], BF16, kind="ExternalInput")
    en_xT = nc.dram_tensor("en_xT", [DIM, S], BF16, kind="ExternalInput")
    wq = nc.dram_tensor("wq", [DIM, HPC * DH], BF16, kind="ExternalInput")
    wk = nc.dram_tensor("wk", [DIM, HPC * DH], BF16, kind="ExternalInput")
    wv = nc.dram_tensor("wv", [DIM, HPC * DH], BF16, kind="ExternalInput")
    wv2 = nc.dram_tensor("wv2", [HPC * DH, DIM], BF16, kind="ExternalInput")
    w1 = nc.dram_tensor("w1", [DIM, DFF], BF16, kind="ExternalInput")
    w2 = nc.dram_tensor("w2", [DFF, DIM], BF16, kind="ExternalInput")
    b1t = nc.dram_tensor("b1t", [128, DFF // 128], F32, kind="ExternalInput")
    b2t = nc.dram_tensor("b2t", [128, DIM // 128], F32, kind="ExternalInput")
    zmask = nc.dram_tensor("zmask", [128, 2], F32, kind="ExternalInput")
    maskT = None
    if has_mask:
        maskT = nc.dram_tensor("maskT", [S, S], BF16, kind="ExternalInput")
    yT = nc.dram_tensor("yT", [DIM, SL], F32, kind="ExternalOutput")

    with tile.TileContext(nc) as tc:
        _trace(nc, tc, plan, de_xT, en_xT, wq, wk, wv, wv2, w1, w2, b1t, b2t,
               zmask, maskT, yT)
    nc.compile()
    return nc, has_mask


def _trace(nc, tc, plan, de_xT, en_xT, wq, wk, wv, wv2, w1, w2, b1t, b2t,
           zmask, maskT, yT):
    # Unified allocation stack: everything (pools and single tiles) must be
    # released in strict LIFO order before TileContext exits.
    stack = nc._tile_stack = []   # [release_fn or None(done)]

    def _push(release_fn):
        ent = {"f": release_fn}
        stack.append(ent)
        def rel():
            assert ent["f"] is not None, "double release"
            ent["f"](); ent["f"] = None
        return rel

    def release_rest():
        for ent in reversed(stack):
            if ent["f"] is not None:
                ent["f"](); ent["f"] = None

    noplan = tuple(tuple('N' for _ in range(NB)) for _ in range(NT))

    # ---- pools ----------------------------------------------------------
    def pool(**kw):
        cm = tc.tile_pool(**kw)
        p = cm.__enter__()
        _push(lambda: cm.__exit__(None, None, None))
        return p

    def single(shape, dtype, name):
        t_, f_ = tc.tile(shape, dtype, name=name)
        return t_, _push(f_)

    has_mask = any(c in 'MF' for row in plan for c in row)
    ps_sc = pool(name="ps_sc", bufs=2, space="PSUM")     # [128,1024] scores/proj
    ps_hx = pool(name="ps_hx", bufs=1, space="PSUM")     # [65,1024] PV accum
    ps_aux = pool(name="ps_aux", bufs=2, space="PSUM")   # [128,512] v2p/FFN
    # pt depth = exp run-ahead across the RS-1 latency gap; the masked build
    # spends 64KB/partition on mask tiles so it gets a shallower pool.
    pt_pool = pool(name="pt", bufs=6 if has_mask else 17)
    rc_pool = pool(name="rc", bufs=1)
    bc_pool = pool(name="bc", bufs=1)
    # bufs=3: q1 pair-0/1 plus the filler-projected q2 pair-0 coexist
    q_pool = pool(name="qp", bufs=3)
    k_pool = pool(name="kp", bufs=3)
    v_pool = pool(name="vp", bufs=2)
    sh_pool = pool(name="shp", bufs=2)
    y_pool = pool(name="yp", bufs=1)
    fold_pool = pool(name="fold", bufs=4)
    dram = pool(name="dram", bufs=1, space="DRAM")

    # ---- persistent tiles (stack order: frees must pop LIFO) ------------
    wq_sb, _ = single([128, KC, HPC * DH], BF16, "wqsb")
    wk_sb, _ = single([128, KC, HPC * DH], BF16, "wksb")
    wv_sb, _ = single([128, KC, HPC * DH], BF16, "wvsb")
    wv2_sb, _ = single([128, 2, DIM], BF16, "wv2sb")
    # warm the ACT exp table while input DMAs stream (allocated below the
    # en/de stack so it is never popped before them)
    warm_sb, _ = single([128, 8], F32, "warm")
    nc.vector.memset(warm_sb[:], 0.0)
    nc.scalar.activation(warm_sb[:], warm_sb[:], AF.Exp)
    # en allocated BELOW de on the stack: de is freed first (after attn1),
    # en later (after attn2 q/k projections).
    en_sb, en_free = [], []
    for kc in range(KC):
        t_, f_ = single([128, S], BF16, f"en{kc}")
        en_sb.append(t_); en_free.append(f_)
    de_sb, de_free = [], []
    for kc in range(KC):
        t_, f_ = single([128, S], BF16, f"de{kc}")
        de_sb.append(t_); de_free.append(f_)
    # DMA trace order: attn1's operands first
    nc.sync.dma_start(wq_sb[:], wq.rearrange("(a p) c -> p a c", p=128))
    nc.sync.dma_start(wk_sb[:], wk.rearrange("(a p) c -> p a c", p=128))
    for kc in range(KC):
        nc.sync.dma_start(de_sb[kc][:], de_xT[kc * 128:(kc + 1) * 128, :])
    nc.sync.dma_start(wv_sb[:], wv.rearrange("(a p) c -> p a c", p=128))
    nc.sync.dma_start(wv2_sb[:], wv2.rearrange("(a p) c -> p a c", p=128))
    for kc in range(KC):
        nc.sync.dma_start(en_sb[kc][:], en_xT[kc * 128:(kc + 1) * 128, :])

    # collective bounce buffers.
    # v2 ReduceScatter, one per s-half: in = 4 chunks (head-groups) of
    # [1024 s, 256 e]; out = this core's summed [1024 s, 256 e].
    ccv_in = [dram.tile([G * 1024, HPC * DH], BF16, name=f"ccvi{t}")
              for t in range(NT)]
    ccv_out = [dram.tile([1024, HPC * DH], BF16, name=f"ccvo{t}")
               for t in range(NT)]
    # per-head h2 AllToAll: in = 8 chunks of [64 f, 512 s]; out = 8 blocks.
    cch_in = [dram.tile([N_CORES * DH, SL], BF16, name=f"cchi{h}")
              for h in range(HPC)]
    cch_out = [dram.tile([N_CORES * DH, SL], BF16, name=f"ccho{h}")
               for h in range(HPC)]

    # ---- helpers --------------------------------------------------------
    # Filler queue: small PE work items drained one per score-block inside
    # attention units. ACT paces attention (~1.1us/block) while PE only needs
    # ~0.85us, so ~1 extra matmul per block rides for free instead of a
    # projection/FFN block stalling the exp pipeline for 7-15us.
    filler = []

    def drain_filler(k):
        for _ in range(k):
            if not filler:
                return
            filler.pop(0)()

    def drain_all():
        while filler:
            filler.pop(0)()

    def project_qk_pair(x_sb, w_sb, pool, pair):
        """q^T (or k^T) for one head pair as a [128, S] bf16 tile."""
        qt = pool.tile([128, S], BF16, tag="qk", name=f"qk{pair}")
        for st in range(NT):
            ps = ps_sc.tile([128, 1024], F32, tag="ps", name="pjps")
            for kc in range(KC):
                for nn in (0, 512):
                    nc.tensor.matmul(
                        ps[:, nn:nn + 512],
                        w_sb[:, kc, pair * 128:(pair + 1) * 128],
                        x_sb[kc][:, st * 1024 + nn:st * 1024 + nn + 512],
                        start=(kc == 0), stop=(kc == KC - 1))
            nc.vector.tensor_copy(qt[:, st * 1024:(st + 1) * 1024], ps[:])
        return qt

    def project_qk_pair_f(x_sb, w_sb, pool, pair):
        """Filler variant: emits the projection as 512-col chunks of 2
        matmuls per item into the aux PSUM pool; returns the tile handle
        immediately (writes land as the filler drains)."""
        qt = pool.tile([128, S], BF16, tag="qk", name=f"qkf{pair}")
        for st in range(NT):
            for nn in (0, 512):
                state = {}
                for kc0 in range(0, KC, 2):
                    def item(st=st, nn=nn, kc0=kc0, state=state):
                        if kc0 == 0:
                            state["ps"] = ps_aux.tile(
                                [128, 512], F32, tag="aps", name="pjf")
                        for kc in (kc0, kc0 + 1):
                            nc.tensor.matmul(
                                state["ps"][:],
                                w_sb[:, kc, pair * 128:(pair + 1) * 128],
                                x_sb[kc][:, st * 1024 + nn:
                                          st * 1024 + nn + 512],
                                start=(kc == 0), stop=(kc == KC - 1))
                    filler.append(item)
                def fin(st=st, nn=nn, state=state):
                    nc.vector.tensor_copy(
                        qt[:, st * 1024 + nn:st * 1024 + nn + 512],
                        state["ps"][:])
                filler.append(fin)
        return qt

    def project_v(src_sb, v_all, blk_lo, blk_hi):
        """v for 4 heads + ones column into v_all[128, NB*HPC*65] (bf16)."""
        for blk in range(blk_lo, blk_hi):
            ps = ps_sc.tile([128, 1024], F32, tag="ps", name="vps")
            for kc in range(KC):
                nc.tensor.matmul(
                    ps[:, 0:HPC * DH],
                    src_sb[kc][:, blk * 128:(blk + 1) * 128],
                    wv_sb[:, kc, :],
                    start=(kc == 0), stop=(kc == KC - 1))
            for h in range(HPC):
                nc.vector.tensor_copy(
                    v_all[:, (blk * HPC + h) * 65:(blk * HPC + h) * 65 + 64],
                    ps[:, h * DH:(h + 1) * DH])

    def new_v_all():
        v_all = v_pool.tile([128, NB * HPC * 65], BF16, tag="v", name="vall")
        v3 = v_all[:].rearrange("p (b c) -> p b c", c=65)
        nc.vector.memset(v3[:, :, 64:65], 1.0)
        return v_all

    def attn_unit(q_pairs, k_pairs, v_all, aplan, mask_tiles_in, sh, h, t):
        """scores -> exp -> PV -> normalize for one (head, query-tile)."""
        pair, off = h // 2, (h % 2) * 64
        hext = ps_hx.tile([65, 1024], F32, name="hext")
        for blk in range(NB):
            sc = ps_sc.tile([128, 1024], F32, tag="ps", name="scps")
            for nn in (0, 512):
                nc.tensor.matmul(
                    sc[:, nn:nn + 512],
                    k_pairs[pair][off:off + 64, blk * 128:(blk + 1) * 128],
                    q_pairs[pair][off:off + 64,
                                  t * 1024 + nn:t * 1024 + nn + 512],
                    start=True, stop=True)
            if aplan[t][blk] == 'M':
                nc.vector.tensor_add(sc[:], sc[:], mask_tiles_in[(t, blk)][:])
            elif aplan[t][blk] == 'F':
                nc.vector.tensor_add(sc[:], sc[:], mask_tiles_in['F'][:])
            pt = pt_pool.tile([128, 1024], BF16, name="pt")
            nc.scalar.activation(pt[:], sc[:], AF.Exp)
            vs = v_all[:, (blk * HPC + h) * 65:(blk * HPC + h) * 65 + 65]
            for nn in (0, 512):
                nc.tensor.matmul(
                    hext[:, nn:nn + 512], vs, pt[:, nn:nn + 512],
                    start=(blk == 0), stop=(blk == NB - 1))
            drain_filler(1)
        recip = rc_pool.tile([1, 1024], F32, name="recip")
        nc.vector.reciprocal(recip[:], hext[64:65, :])
        rbc = bc_pool.tile([64, 1024], F32, name="rbc")
        nc.gpsimd.partition_broadcast(rbc[:], recip[0:1, :])
        nc.vector.tensor_mul(
            sh[pair][off:off + 64, t * 1024:(t + 1) * 1024],
            hext[0:64, :], rbc[:])

    def new_sh():
        return [sh_pool.tile([128, S], BF16, tag="sh", name=f"sh{p}")
                for p in range(2)]

    def v2_partial_half(h1sh, t):
        """Partial v2 (own 256 h1 features x all 1024 v-cols) for s-half t,
        DMA'd into the RS bounce buffer as 4 head-group chunks."""
        for sb in range(8):
            col = t * 1024 + sb * 128
            vp = fold_pool.tile([128, DIM], BF16, tag="v2p", name="v2p")
            for eh in (0, 512):
                ps = ps_aux.tile([128, 512], F32, tag="aps", name="v2ps")
                for pair in range(2):
                    nc.tensor.matmul(
                        ps[:],
                        h1sh[pair][:, col:col + 128],
                        wv2_sb[:, pair, eh:eh + 512],
                        start=(pair == 0), stop=(pair == 1))
                # DVE, not Pool: the q2/k2 projection copies ride the Pool
                # queue and must not stall behind these
                nc.vector.tensor_copy(vp[:, eh:eh + 512], ps[:])
            for g in range(G):
                nc.sync.dma_start(
                    ccv_in[t][g * 1024 + sb * 128:g * 1024 + (sb + 1) * 128, :],
                    vp[:, g * HPC * DH:(g + 1) * HPC * DH])
        nc.gpsimd.collective_compute(
            "ReduceScatter", mybir.AluOpType.add,
            replica_groups=[[0, 1, 2, 3], [4, 5, 6, 7]],
            ins=[ccv_in[t].opt()], outs=[ccv_out[t].opt()])

    def v2_fill_half(v_all, t):
        """DMA the reduce-scattered v2 s-half into v_all's per-head slots."""
        v3 = v_all[:].rearrange("p (b h c) -> p b h c", h=HPC, c=65)
        for sb in range(8):
            blk = t * 8 + sb
            nc.sync.dma_start(
                v3[:, blk, :, 0:64],
                ccv_out[t][sb * 128:(sb + 1) * 128, :]
                .rearrange("p (h c) -> p h c", c=64))

    # ---- attention 1 (self-attn on de_x, mask) --------------------------
    # pair-0 q/k and v project first so the exp pipeline (ACT) starts as
    # early as possible; pair-1 projections slot in behind the first units.
    q1 = [None, None]
    k1 = [None, None]
    q1[0] = project_qk_pair(de_sb, wq_sb, q_pool, 0)
    k1[0] = project_qk_pair(de_sb, wk_sb, k_pool, 0)
    v1 = new_v_all()
    project_v(de_sb, v1, 0, NB)
    mask_tiles, mask_free = {}, []
    if any(c == 'F' for row in plan for c in row):
        ft, ff = single([128, 1024], BF16, "mkF")
        nc.vector.memset(ft[:], NEG)
        mask_tiles['F'] = ft
        mask_free.append(ff)
    for t in range(NT):
        for blk in range(NB):
            if plan[t][blk] == 'M':
                mt, fm = single([128, 1024], BF16, f"mk{t}_{blk}")
                nc.sync.dma_start(
                    mt[:], maskT[blk * 128:(blk + 1) * 128,
                                 t * 1024:(t + 1) * 1024])
                mask_tiles[(t, blk)] = mt
                mask_free.append(fm)
    # t-major so each s-half of the partial-v2 ReduceScatter fires while the
    # other half's attention units are still computing.
    h1sh = new_sh()
    v2 = new_v_all()
    q2 = [None, None]
    k2 = [None, None]
    for t in range(NT):
        for h in range(HPC):
            if t == 0 and h == 0:
                # pair-1 projections ride t0's ACT slack via the filler
                q1[1] = project_qk_pair_f(de_sb, wq_sb, q_pool, 1)
                k1[1] = project_qk_pair_f(de_sb, wk_sb, k_pool, 1)
            if t == 0 and h == 2:
                drain_all()   # pair-1 q/k must be fully written before use
            # attn2 pair-0 projections ride t1's slack so exp2 can start
            # the moment attention 1 drains (hiding RS-1)
            if t == 1 and h == 0:
                q2[0] = project_qk_pair_f(en_sb, wq_sb, q_pool, 0)
                k2[0] = project_qk_pair_f(en_sb, wk_sb, k_pool, 0)
            attn_unit(q1, k1, v1, plan, mask_tiles, h1sh, h, t)
        drain_all()
        if t == 0:
            v2_partial_half(h1sh, 0)
    for f in reversed(mask_free):
        f()
    for f in reversed(de_free):
        f()

    # ---- attention 2 (q,k from en_x; v from reduce-scattered h1@Wv) -----
    # Order on PE after the last attn1 unit: t=1 v2 partials (starts RS-1
    # early); the pair-1 projections ride attn2-h0's units via the filler.
    v2_partial_half(h1sh, 1)
    v2_fill_half(v2, 0)
    # pair-1 projections ride attn2-h0's units via the filler; the en frees
    # and the weight-tile creations that must follow them are deferred into
    # the loop at h==1 (after those filler items have drained).
    q2[1] = project_qk_pair_f(en_sb, wq_sb, q_pool, 1)
    k2[1] = project_qk_pair_f(en_sb, wk_sb, k_pool, 1)
    v2_fill_half(v2, 1)

    w1_sb, w2_sb, ff1_sb = [], [], []
    F = {}

    def setup_ffn_weights():
        # w1 is host-permuted so per-head folded A2A chunks hit contiguous
        # 128-row stationary slices
        for kc in range(KC):
            t_, _ = single([128, DFF], BF16, f"w1_{kc}")
            nc.sync.dma_start(t_[:], w1[kc * 128:(kc + 1) * 128, :])
            w1_sb.append(t_)
        F['b1'], _ = single([128, DFF // 128], F32, "b1sb")
        F['b2'], _ = single([128, DIM // 128], F32, "b2sb")
        F['zm'], _ = single([128, 2], F32, "zmsb")
        nc.sync.dma_start(F['b1'][:], b1t[:])
        nc.sync.dma_start(F['b2'][:], b2t[:])
        nc.sync.dma_start(F['zm'][:], zmask[:])
        for dc in range(DFF // 128):
            t_, _ = single([128, DIM], BF16, f"w2_{dc}")
            nc.sync.dma_start(t_[:], w2[dc * 128:(dc + 1) * 128, :])
            w2_sb.append(t_)
        for dffb in range(DFF // 128):
            t_, _ = single([128, SL], BF16, f"ff1_{dffb}")
            ff1_sb.append(t_)

    def ffn_fold(h):
        """Fold head h's A2A arrival (zmask zeroes the foreign-batch copy).
        Recv DMAs ride the Pool queue so they never head-of-line-block the
        SP queue behind a later head's A2A input DMAs."""
        fold = []
        for i in range(2):
            lo = fold_pool.tile([128, SL], BF16, tag="fl", name=f"flo{h}_{i}")
            hi = fold_pool.tile([128, SL], BF16, tag="fh", name=f"fhi{h}_{i}")
            nc.gpsimd.dma_start(lo[:], cch_out[h][i * 128:(i + 1) * 128, :])
            nc.gpsimd.dma_start(
                hi[:], cch_out[h][256 + i * 128:256 + (i + 1) * 128, :])
            nc.vector.tensor_scalar_mul(lo[:], lo[:], F['zm'][:, 0:1])
            nc.vector.tensor_scalar_mul(hi[:], hi[:], F['zm'][:, 1:2])
            nc.vector.tensor_add(lo[:], lo[:], hi[:])
            fold.append(lo)
        return fold

    def ffn_w1(h, tail):
        """Head h's W1 contribution, accumulated in-place in ff1_sb.
        tail=False emits via the filler (safe: only queued two heads after
        the A2A fired, so the data is long since landed)."""
        fold = ffn_fold(h)
        for dffb in range(DFF // 128):
            state = {}
            def it_mm(h=h, dffb=dffb, fold=fold, state=state):
                ps = ps_aux.tile([128, 512], F32, tag="aps", name="f1ps")
                state["ps"] = ps
                for i in range(2):
                    nc.tensor.matmul(
                        ps[:, 0:SL],
                        w1_sb[2 * h + i][:, dffb * 128:(dffb + 1) * 128],
                        fold[i][:],
                        start=(i == 0), stop=(i == 1))
            def it_acc(h=h, dffb=dffb, state=state):
                ps = state["ps"]
                if h == 0:
                    nc.vector.tensor_copy(ff1_sb[dffb][:], ps[:, 0:SL])
                elif h < HPC - 1:
                    nc.vector.tensor_add(ff1_sb[dffb][:], ff1_sb[dffb][:],
                                         ps[:, 0:SL])
                else:
                    nc.vector.tensor_add(ps[:, 0:SL], ps[:, 0:SL],
                                         ff1_sb[dffb][:])
                    nc.scalar.activation(ff1_sb[dffb][:], ps[:, 0:SL],
                                         AF.Relu,
                                         bias=F['b1'][:, dffb:dffb + 1])
            if tail:
                it_mm(); it_acc()
            else:
                filler.append(it_mm); filler.append(it_acc)

    # h-major so each head's A2A fires while later heads are still
    # computing; head h-2's fold+W1 pass rides the filler through head h's
    # units (two heads back => its A2A has long completed).
    h2sh = new_sh()
    for h in range(HPC):
        if h == 1:
            # pair-1 q2/k2 filler items (which read en) have drained during
            # h0's units; now en can be freed and its space reused for the
            # FFN weights.
            drain_all()
            for f in reversed(en_free):
                f()
            setup_ffn_weights()
        if h >= 2:
            ffn_w1(h - 2, tail=False)
        for t in range(NT):
            attn_unit(q2, k2, v2, noplan, {}, h2sh, h, t)
        pair, off = h // 2, (h % 2) * 64
        for j in range(N_CORES):
            nc.sync.dma_start(
                cch_in[h][j * DH:(j + 1) * DH, :],
                h2sh[pair][off:off + 64, (j % G) * SL:(j % G + 1) * SL])
        nc.gpsimd.collective_compute(
            "AllToAll", mybir.AluOpType.bypass,
            replica_groups=[[0, 1, 2, 3, 4, 5, 6, 7]],
            ins=[cch_in[h].opt()], outs=[cch_out[h].opt()])

    # ---- FFN tail on own S/4 rows --------------------------------------
    drain_all()
    ffn_w1(HPC - 2, tail=True)
    ffn_w1(HPC - 1, tail=True)
    for dimb in range(DIM // 128):
        ps = ps_aux.tile([128, 512], F32, tag="aps", name="yps")
        for dc in range(DFF // 128):
            nc.tensor.matmul(
                ps[:, 0:SL],
                w2_sb[dc][:, dimb * 128:(dimb + 1) * 128],
                ff1_sb[dc][:],
                start=(dc == 0), stop=(dc == DFF // 128 - 1))
        ysb = y_pool.tile([128, SL], F32, tag="y", name="ysb")
        nc.vector.tensor_scalar_add(ysb[:], ps[:, 0:SL],
                                    F['b2'][:, dimb:dimb + 1])
        nc.sync.dma_start(yT[dimb * 128:(dimb + 1) * 128, :], ysb[:])

    release_rest()


def _prep_inputs(de_x, en_x, mask, Wq, Wk, Wv, W1, b1, W2, b2, has_mask):
    bf = ml_dtypes.bfloat16
    scale = 1.0 / math.sqrt(DH)
    in_maps = []
    deT = [np.ascontiguousarray(de_x[b].T).astype(bf) for b in range(B)]
    enT = [np.ascontiguousarray(en_x[b].T).astype(bf) for b in range(B)]
    # W1 rows permuted so that per-head A2A fold chunks are contiguous:
    # w1p[h*256 + j*64 + r] = W1[(j*4 + h)*64 + r]
    perm = np.empty(DIM, np.int64)
    for h in range(HPC):
        for j in range(G):
            perm[h * 256 + j * 64:h * 256 + (j + 1) * 64] = \
                np.arange((j * G + h) * 64, (j * G + h) * 64 + 64)
    w1p = np.ascontiguousarray(W1[perm]).astype(bf)
    w2b = W2.astype(bf)
    b1t = np.ascontiguousarray(b1.reshape(DFF // 128, 128).T).astype(np.float32)
    b2t = np.ascontiguousarray(b2.reshape(DIM // 128, 128).T).astype(np.float32)
    wv_flat = np.ascontiguousarray(
        np.transpose(Wv, (1, 0, 2)).reshape(DIM, H * DH)).astype(bf)
    mT = None
    if has_mask:
        mT = np.ascontiguousarray(mask.T * np.float32(NEG)).astype(bf)
    for c in range(N_CORES):
        b, g = divmod(c, G)
        hs = slice(g * HPC, (g + 1) * HPC)
        m = {
            "de_xT": deT[b],
            "en_xT": enT[b],
            "wq": np.ascontiguousarray(
                np.transpose(Wq[hs] * scale, (1, 0, 2)).reshape(DIM, HPC * DH)
            ).astype(bf),
            "wk": np.ascontiguousarray(
                np.transpose(Wk[hs], (1, 0, 2)).reshape(DIM, HPC * DH)).astype(bf),
            "wv": np.ascontiguousarray(
                np.transpose(Wv[hs], (1, 0, 2)).reshape(DIM, HPC * DH)).astype(bf),
            # partial-v2 weights: rows = this core's 256 h1-features,
            # cols = all 16 heads' v outputs
            "wv2": np.ascontiguousarray(
                wv_flat[g * HPC * DH:(g + 1) * HPC * DH, :]),
            "w1": w1p, "w2": w2b, "b1t": b1t, "b2t": b2t,
        }
        zm = np.zeros((128, 2), np.float32)
        zm[:, 0 if b == 0 else 1] = 1.0
        m["zmask"] = zm
        if has_mask:
            m["maskT"] = mT
        in_maps.append(m)
    return in_maps


def get_program(mask):
    plan = _mask_plan(np.asarray(mask))
    if plan not in _CACHE:
        _CACHE[plan] = _build(plan)
    return _CACHE[plan]


_RUNNERS: dict = {}


def _fast_runner(nc):
    """Build (once) a cached jitted SPMD executor for this program.

    run_bass_kernel_spmd re-creates and re-traces its jax.jit closure on
    every call; caching the jitted shard_map shaves seconds of dispatch
    overhead off warm calls. Mirrors bass2jax.run_bass_via_pjrt.
    """
    import jax
    from jax.sharding import Mesh, PartitionSpec
    try:
        from jax.experimental.shard_map import shard_map
    except ImportError:
        from jax.shard_map import shard_map
    import concourse.mybir as _mb
    from concourse import bass2jax as b2j

    b2j.install_neuronx_cc_hook()
    partition_name = (nc.partition_id_tensor.name
                      if nc.partition_id_tensor else None)
    in_names, out_names, out_avals = [], [], []
    for alloc in nc.m.functions[0].allocations:
        if not isinstance(alloc, _mb.MemoryLocationSet):
            continue
        name = alloc.memorylocations[0].name
        if alloc.kind == "ExternalInput":
            if name != partition_name:
                in_names.append(name)
        elif alloc.kind == "ExternalOutput":
            out_names.append(name)
            out_avals.append(jax.core.ShapedArray(
                tuple(alloc.tensor_shape), _mb.dt.np(alloc.dtype)))
    n_params = len(in_names)
    n_outs = len(out_avals)
    all_names = in_names + out_names + ([partition_name] if partition_name else [])
    donate = tuple(range(n_params, n_params + n_outs))

    def _body(*args):
        operands = list(args)
        if partition_name is not None:
            operands.append(b2j.partition_id_tensor())
        return tuple(b2j._bass_exec_p.bind(
            *operands,
            out_avals=tuple(out_avals),
            in_names=tuple(all_names),
            out_names=tuple(out_names),
            lowering_input_output_aliases=(),
            sim_require_finite=True,
            sim_require_nnan=True,
            nc=nc,
        ))

    devices = jax.devices()[:N_CORES]
    mesh = Mesh(np.asarray(devices), ("core",))
    in_specs = (PartitionSpec("core"),) * (n_params + n_outs)
    out_specs = (PartitionSpec("core"),) * n_outs
    sharded = jax.jit(
        shard_map(_body, mesh=mesh, in_specs=in_specs, out_specs=out_specs,
                  check_rep=False),
        donate_argnums=donate, keep_unused=True)

    def runner(in_maps):
        concat_in = [np.concatenate([in_maps[c][nm] for c in range(N_CORES)],
                                    axis=0) for nm in in_names]
        zeros = [np.zeros((N_CORES * a.shape[0], *a.shape[1:]), a.dtype)
                 for a in out_avals]
        out_arrs = sharded(*concat_in, *zeros)
        return [
            {nm: np.asarray(out_arrs[i]).reshape(N_CORES, *out_avals[i].shape)[c]
             for i, nm in enumerate(out_names)}
            for c in range(N_CORES)
        ]

    return runner


def run(inputs, want_results=False, **run_kwargs):
    nc, has_mask = get_program(inputs["mask"])
    in_maps = _prep_inputs(
        inputs["de_x"], inputs["en_x"], inputs["mask"],
        inputs["Wq"], inputs["Wk"], inputs["Wv"],
        inputs["W1"], inputs["b1"], inputs["W2"], inputs["b2"], has_mask)
    results = None
    res = None
    if not run_kwargs:
        try:
            key = id(nc)
            if key not in _RUNNERS:
                _RUNNERS[key] = _fast_runner(nc)
            results = _RUNNERS[key](in_maps)
        except Exception:
            results = None
    if results is None:
        res = bass_utils.run_bass_kernel_spmd(
            nc, in_maps, core_ids=list(range(N_CORES)), **run_kwargs)
        results = res.results
    y = np.empty((B, S, DIM), np.float32)
    for c in range(N_CORES):
        b, g = divmod(c, G)
        y[b, g * SL:(g + 1) * SL, :] = results[c]["yT"].T
    return (y, res) if want_results else y


def kernel(**inputs) -> np.ndarray:
    return run({k: np.asarray(v) for k, v in inputs.items()})
